# revision 12
# baseline (speedup 1.0000x reference)
"""MPN-COV pooling + projection kernel for 8 Trainium2 NeuronCores.

Problem: nn_PillTeacher_48661979464182
  feat [64, 256, 14, 14] -> per-sample covariance + 3 Newton-Schulz sqrt
  iterations -> L2-normalize -> project with W_proj [512, 65536] -> BN -> L2.

Sharding:
  - Pooling phase: pure data parallel, 8 samples per core.
  - Projection: k-shard of W_proj (each core holds an 8192-wide slice of the
    contraction dim). AllToAll exchanges the normalized pooled matrices so
    every core gets its k-slice of all 64 samples; partial embeddings are
    summed with ReduceScatter back to the owning core of each sample.

Key tricks:
  - Every matrix in the Newton-Schulz iteration is a polynomial of the
    (symmetric) covariance -> symmetric -> matmul lhsT operands read the
    row-major tiles directly (no transposes on device; feat pre-transposed
    on host).
  - The final L2 normalization is invariant to any positive per-sample
    scale, so 1/M, 1/trace, sqrt(trY) and the 0.5 of the last NS Y-update
    all drop out.
  - fp32r (4x-rate fp32 matmul mode) for all matmuls.
  - BN + bias folded into a host-computed scale/shift.

Workarounds for this walrus build:
  - <=1 semaphore wait per instruction (_split_excess_waits post-pass).
  - no matmul with rhs free size 1 (scalar reductions go through [1, 256]
    row-sums + a free-axis reduce; scalar broadcasts use [1, 2] operands).
  - no tensor_tensor_reduce (mask-mult + tensor_reduce / activation instead).
"""
import sys
import numpy as np

sys.path.insert(0, "/opt/trn_rl_repo")

import concourse.bass as bass
import concourse.mybir as mybir
import concourse.tile as tile
import bass_rust
from concourse.bass_utils import run_bass_kernel_spmd

dt = mybir.dt

N_CORES = 8
B, C, H, W_SP = 64, 256, 14, 14
M = H * W_SP           # 196
E = 512
K = C * C              # 65536
BL = B // N_CORES      # 8 samples per core
KL = K // N_CORES      # 8192 contraction slice per core
BN_EPS = 1e-5

_cache = {}


def _split_excess_waits(nc, max_waits=1):
    """walrus in this env rejects >1 semaphore wait per instruction; hoist
    excess waits onto preceding NoOps on the same engine."""
    for fn in nc.m.functions:
        for bb in fn.blocks:
            new_insts = []
            for inst in bb.instructions:
                si = inst.sync_info
                if si is not None and si.on_wait and len(si.on_wait) > max_waits:
                    waits = list(si.on_wait)
                    chunks = [waits[i:i + max_waits]
                              for i in range(0, len(waits), max_waits)]
                    for chunk in chunks[:-1]:
                        nop = mybir.InstNoOp(
                            name=nc.get_next_instruction_name(), ins=[], outs=[],
                            engine=inst.engine)
                        nop.sync_info = bass_rust.SyncInfo(on_wait=chunk,
                                                           on_update=[])
                        new_insts.append(nop)
                    si.on_wait = chunks[-1]
                new_insts.append(inst)
            bb.instructions = new_insts


def _build(stage=5):
    """stage: 1=Y0 dump, 2=F dump, 3=a2a_out dump, 4=emb partial dump,
    5=full kernel.

    All 256x256 matrices use a stacked-pair layout: S[p, r*256 + j] =
    X[128*r + p, j] -- one [128, 512] tile per matrix, so every elementwise
    op is a single instruction and every PSUM product fills one full bank."""
    f32, f32r = dt.float32, dt.float32r
    nc = bass.Bass("TRN2", target_bir_lowering=False, debug=False,
                   num_devices=N_CORES)

    featT = nc.dram_tensor("featT", [BL, M, C], f32r, kind="ExternalInput")
    onesc = nc.dram_tensor("onesc", [128, 1], f32r, kind="ExternalInput")
    onesr = nc.dram_tensor("onesr", [1, 128], f32r, kind="ExternalInput")
    ident = nc.dram_tensor("identS", [128, 2 * C], f32, kind="ExternalInput")
    if stage >= 4:
        wT = nc.dram_tensor("wT", [KL, E], f32r, kind="ExternalInput")
    if stage >= 5:
        bnsc = nc.dram_tensor("bnsc", [BL, E], f32, kind="ExternalInput")
        bnsh = nc.dram_tensor("bnsh", [BL, E], f32, kind="ExternalInput")
        out = nc.dram_tensor("out", [BL, E], f32, kind="ExternalOutput")
    elif stage <= 2:
        out = nc.dram_tensor("dbg", [2, 128, C], f32, kind="ExternalOutput")
    elif stage == 3:
        out = nc.dram_tensor("dbg", [128, 4096], f32, kind="ExternalOutput")
    else:
        out = nc.dram_tensor("dbg", [64, E], f32, kind="ExternalOutput")

    rg = [list(range(N_CORES))]
    AluOp = mybir.AluOpType
    NCH = KL // 128        # 64 k-chunks for the projection

    lp = nc.allow_low_precision(reason="f32r intermediates carry fp32 bits")
    lp.__enter__()
    with tile.TileContext(nc) as tc:
        with (
            tc.tile_pool(name="consts", bufs=1) as cpool,
            tc.tile_pool(name="wbuf", bufs=1) as wpool,
            tc.tile_pool(name="big", bufs=1) as bigpool,
            tc.tile_pool(name="work", bufs=3) as pool,
            tc.tile_pool(name="mats", bufs=2) as mats,
            tc.tile_pool(name="pss", bufs=2, space="PSUM") as pss,
            tc.tile_pool(name="psg", bufs=3, space="PSUM") as psg,
            tc.tile_pool(name="pse", bufs=1, space="PSUM") as pse,
            tc.tile_pool(name="dram", bufs=1, space="DRAM") as dram,
        ):
            # ---------- constants ----------
            ones_t = cpool.tile([128, 1], f32r, name="ones_t")
            nc.sync.dma_start(ones_t[:], onesc[:])
            onesr_t = cpool.tile([1, 128], f32r, name="onesr_t")
            nc.sync.dma_start(onesr_t[:], onesr[:])
            identS_t = cpool.tile([128, 2 * C], f32, name="identS_t")
            threeIS_t = cpool.tile([128, 2 * C], f32, name="threeIS_t")
            nc.sync.dma_start(identS_t[:], ident[:])
            nc.scalar.mul(threeIS_t[:], identS_t[:], 3.0)
            if stage >= 5:
                bnsc_t = cpool.tile([BL, E], f32, name="bnsc_t")
                bnsh_t = cpool.tile([BL, E], f32, name="bnsh_t")
                nc.sync.dma_start(bnsc_t[:], bnsc[:])
                nc.sync.dma_start(bnsh_t[:], bnsh[:])

            # ---------- W prefetch: [KL, E] -> SBUF [128, NCH*E] ----------
            if stage >= 4:
                Wq = wpool.tile([128, NCH * E], f32r, name="Wq")
                wT_v = wT.rearrange("(g p) e -> p g e", p=128)   # [128,64,512]
                for g in range(8):
                    nc.sync.dma_start(
                        Wq[:, g * 8 * E:(g + 1) * 8 * E]
                            .rearrange("p (c e) -> p c e", c=8),
                        wT_v[:, 8 * g:8 * (g + 1), :])

            # ---------- DRAM staging for collectives ----------
            if stage >= 3:
                # a2a flat layout: [j(8), h(2), p(128), b_l(8), i0(32)]
                a2a_in = dram.tile([128, 4096], f32r, name="a2a_in")
                a2a_out = dram.tile([128, 4096], f32r, name="a2a_out")
                a2a_in_v = a2a_in.flatten().rearrange(
                    "(j h p b i) -> h b p j i", j=8, h=2, p=128, b=BL, i=32)
            if stage >= 5:
                rs_in = dram.tile([B, E], f32, name="rs_in")
                rs_out = dram.tile([BL, E], f32, name="rs_out")

            def mmp(outS, AS, BS):
                """outS = A @ B for symmetric A, all in stacked-pair layout."""
                for r in range(2):
                    for kc in range(2):
                        nc.tensor.matmul(
                            outS[:, C * r:C * (r + 1)],
                            AS[:, C * kc + 128 * r:C * kc + 128 * r + 128],
                            BS[:, C * kc:C * (kc + 1)],
                            start=(kc == 0), stop=(kc == 1))

            def scalar_bcast(val_sb, tag):
                """[1,1] f32r scalar -> [128,1] f32 SBUF (via N=2 matmul)."""
                v2 = pool.tile([1, 2], f32r, name=f"v2{tag}", tag=f"v2{tag}")
                nc.vector.tensor_copy(v2[:, 0:1], val_sb[:])
                nc.vector.tensor_copy(v2[:, 1:2], val_sb[:])
                b_ps = pss.tile([128, 2], f32, name=f"bps{tag}", tag="sm2")
                nc.tensor.matmul(b_ps[:], onesr_t[:], v2[:],
                                 start=True, stop=True)
                b_sb = pool.tile([128, 1], f32, name=f"bsb{tag}", tag=f"bsb{tag}")
                nc.vector.tensor_copy(b_sb[:], b_ps[:, 0:1])
                return b_sb

            # ---------- pooling phase: BL samples ----------
            nsamp = 1 if stage <= 2 else BL
            for b in range(nsamp):
                B0 = pool.tile([128, C], f32r, name="B0", tag="B0")
                B1 = pool.tile([M - 128, C], f32r, name="B1", tag="B1")
                nc.sync.dma_start(B0[:], featT[b, 0:128, :])
                nc.sync.dma_start(B1[:], featT[b, 128:M, :])

                # column sums -> [1, 256]
                srow_ps = pss.tile([1, 2 * C], f32, name="srow", tag="sm1")
                nc.tensor.matmul(srow_ps[:, 0:C], ones_t[0:128, :], B0[:],
                                 start=True, stop=False)
                nc.tensor.matmul(srow_ps[:, 0:C], ones_t[0:M - 128, :], B1[:],
                                 start=False, stop=True)
                s_sb = pool.tile([1, C], f32r, name="s_sb", tag="s_sb", bufs=2)
                t_sb = pool.tile([1, C], f32r, name="t_sb", tag="t_sb", bufs=2)
                nc.scalar.copy(s_sb[:], srow_ps[:, 0:C])
                nc.scalar.mul(t_sb[:], srow_ps[:, 0:C], -1.0 / M)

                # G = A^T A - M xbar xbar^T  (stacked PSUM [128, 512])
                GS = psg.tile([128, 2 * C], f32, name="GS", tag="Yp")
                for r in range(2):
                    nc.tensor.matmul(GS[:, C * r:C * (r + 1)],
                                     B0[:, 128 * r:128 * (r + 1)], B0[:],
                                     start=True, stop=False)
                    nc.tensor.matmul(GS[:, C * r:C * (r + 1)],
                                     B1[:, 128 * r:128 * (r + 1)], B1[:],
                                     start=False, stop=False)
                    nc.tensor.matmul(GS[:, C * r:C * (r + 1)],
                                     t_sb[:, 128 * r:128 * (r + 1)], s_sb[:],
                                     start=False, stop=True)

                # trace: diag mask -> column sums -> free-axis reduce
                scrS = pool.tile([128, 2 * C], f32r, name="scrS", tag="scr", bufs=2)
                nc.vector.tensor_tensor(scrS[:], GS[:], identS_t[:],
                                        AluOp.mult)
                trrow_ps = pss.tile([1, 2 * C], f32, name="trrow", tag="sm1")
                nc.tensor.matmul(trrow_ps[:], ones_t[0:128, :], scrS[:],
                                 start=True, stop=True)
                tr_sb = pool.tile([1, 1], f32, name="tr_sb", tag="tr_sb")
                nc.vector.tensor_reduce(out=tr_sb[:], in_=trrow_ps[:],
                                        axis=mybir.AxisListType.X,
                                        op=AluOp.add)
                inv_sb = pool.tile([1, 1], f32r, name="inv_sb", tag="inv")
                nc.vector.reciprocal(inv_sb[:], tr_sb[:])
                invb = scalar_bcast(inv_sb, "i")

                # Y0 = G / trG
                Y0S = mats.tile([128, 2 * C], f32r, name="Y0S", tag="Y0")
                nc.vector.tensor_scalar_mul(Y0S[:], GS[:], invb[:])

                if stage == 1:
                    for r in range(2):
                        nc.sync.dma_start(out[r, :, :],
                                          Y0S[:, C * r:C * (r + 1)].bitcast(f32))
                    break

                # ---- NS iter 1 (Z0=I): T1 = 3I - Y0; Y1 = .5 Y0 T1; Z1 = .5 T1
                T1S = mats.tile([128, 2 * C], f32r, name="T1S", tag="T")
                nc.vector.scalar_tensor_tensor(
                    out=T1S[:], in0=Y0S[:], scalar=-1.0, in1=threeIS_t[:],
                    op0=AluOp.mult, op1=AluOp.add)
                YpS = psg.tile([128, 2 * C], f32, name="YpS", tag="Yp")
                mmp(YpS, Y0S, T1S)
                Y1S = mats.tile([128, 2 * C], f32r, name="Y1S", tag="Y1")
                Z1S = mats.tile([128, 2 * C], f32r, name="Z1S", tag="Z")
                nc.scalar.mul(Y1S[:], YpS[:], 0.5)
                nc.scalar.mul(Z1S[:], T1S[:], 0.5)

                # ---- NS iter 2
                PpS = psg.tile([128, 2 * C], f32, name="PpS", tag="Yp")
                mmp(PpS, Z1S, Y1S)
                T2S = mats.tile([128, 2 * C], f32r, name="T2S", tag="T")
                nc.vector.scalar_tensor_tensor(
                    out=T2S[:], in0=PpS[:], scalar=-1.0, in1=threeIS_t[:],
                    op0=AluOp.mult, op1=AluOp.add)
                Yp2S = psg.tile([128, 2 * C], f32, name="Yp2S", tag="Yp")
                mmp(Yp2S, Y1S, T2S)
                Y2S = mats.tile([128, 2 * C], f32r, name="Y2S", tag="Y2")
                nc.scalar.mul(Y2S[:], Yp2S[:], 0.5)
                ZpS = psg.tile([128, 2 * C], f32, name="ZpS", tag="Yp")
                mmp(ZpS, T2S, Z1S)
                Z2S = mats.tile([128, 2 * C], f32r, name="Z2S", tag="Z")
                nc.scalar.mul(Z2S[:], ZpS[:], 0.5)

                # ---- NS iter 3 (Z dead): Y3 = Y2 (3I - Z2 Y2), unscaled
                Pp3S = psg.tile([128, 2 * C], f32, name="Pp3S", tag="Yp")
                mmp(Pp3S, Z2S, Y2S)
                T3S = mats.tile([128, 2 * C], f32r, name="T3S", tag="T")
                nc.vector.scalar_tensor_tensor(
                    out=T3S[:], in0=Pp3S[:], scalar=-1.0, in1=threeIS_t[:],
                    op0=AluOp.mult, op1=AluOp.add)
                Y3pS = psg.tile([128, 2 * C], f32, name="Y3pS", tag="Yp")
                mmp(Y3pS, Y2S, T3S)

                # ---- flat-normalize: F = Y3 / ||Y3||_F
                sqS = pool.tile([128, 2 * C], f32r, name="sqS", tag="scr", bufs=2)
                nc.scalar.activation(sqS[:], Y3pS[:],
                                     mybir.ActivationFunctionType.Square)
                ssqrow_ps = pss.tile([1, 2 * C], f32, name="ssqrow", tag="sm1")
                nc.tensor.matmul(ssqrow_ps[:], ones_t[0:128, :], sqS[:],
                                 start=True, stop=True)
                ssq_sb = pool.tile([1, 1], f32, name="ssq_sb", tag="tr_sb")
                nc.vector.tensor_reduce(out=ssq_sb[:], in_=ssqrow_ps[:],
                                        axis=mybir.AxisListType.X,
                                        op=AluOp.add)
                sqr_sb = pool.tile([1, 1], f32, name="sqr_sb", tag="sqr")
                nc.scalar.sqrt(sqr_sb[:], ssq_sb[:])
                rsq_sb = pool.tile([1, 1], f32r, name="rsq_sb", tag="inv")
                nc.vector.reciprocal(rsq_sb[:], sqr_sb[:])
                rsqb = scalar_bcast(rsq_sb, "r")
                FS = mats.tile([128, 2 * C], f32r, name="FS", tag="F")
                nc.vector.tensor_scalar_mul(FS[:], Y3pS[:], rsqb[:])

                if stage == 2:
                    for r in range(2):
                        nc.sync.dma_start(out[r, :, :],
                                          FS[:, C * r:C * (r + 1)].bitcast(f32))
                    break

                # ---- staging: dest j gets F[:, 32j:32j+32] (symmetry!)
                for hh in range(2):
                    nc.sync.dma_start(
                        a2a_in_v[hh, b],                          # [p, j, i]
                        FS[:, C * hh:C * (hh + 1)]
                            .rearrange("p (j i) -> p j i", j=8, i=32))

            # ---------- AllToAll ----------
            if stage >= 3:
                nc.gpsimd.collective_compute(
                    "AllToAll", AluOp.bypass, replica_groups=rg,
                    ins=[a2a_in.opt()], outs=[a2a_out.opt()])

            if stage == 3:
                tmp = bigpool.tile([128, 4096], f32, name="tmp")
                nc.sync.dma_start(tmp[:], a2a_out[:].bitcast(f32))
                nc.sync.dma_start(out[:], tmp[:])

            if stage >= 4:
                # ------- consumer: BIG [128, 4096], free = [h, s, b, i] -------
                BIG = bigpool.tile([128, 2 * 8 * BL * 32], f32r, name="BIG")
                a2a_out_v = a2a_out.flatten().rearrange(
                    "(s h p b i) -> h s p b i", s=8, h=2, p=128, b=BL, i=32)
                BIG_v = BIG[:].rearrange("p (h s b i) -> h s p b i",
                                         h=2, s=8, b=BL, i=32)
                for hh in range(2):
                    for s in range(8):
                        nc.sync.dma_start(BIG_v[hh, s], a2a_out_v[hh, s])

                # ------- projection: EMB[64, 512] -------
                EMB = pse.tile([64, E], f32, name="EMB")
                BIG_k = BIG[:].rearrange("p (h sb i) -> h i p sb",
                                         h=2, sb=64, i=32)
                for c in range(NCH):
                    i_local, hh = c // 2, c % 2
                    nc.tensor.matmul(
                        EMB[:], BIG_k[hh, i_local], Wq[:, E * c:E * (c + 1)],
                        start=(c == 0), stop=(c == NCH - 1))

                emb_sb = pool.tile([64, E], f32, name="emb_sb", tag="emb", bufs=1)
                nc.vector.tensor_copy(emb_sb[:], EMB[:])
                if stage == 4:
                    nc.sync.dma_start(out[:], emb_sb[:])

            if stage >= 5:
                nc.sync.dma_start(rs_in[:], emb_sb[:])

                # ------- ReduceScatter: [64, E] -> [8, E] -------
                nc.gpsimd.collective_compute(
                    "ReduceScatter", AluOp.add, replica_groups=rg,
                    ins=[rs_in.opt()], outs=[rs_out.opt()])

                # ------- BN fold + final L2 normalize -------
                e_sb = pool.tile([BL, E], f32, name="e_sb", tag="fin", bufs=1)
                nc.sync.dma_start(e_sb[:], rs_out[:])
                e_bn = pool.tile([BL, E], f32, name="e_bn", tag="fin2", bufs=1)
                nc.vector.tensor_tensor(e_bn[:], e_sb[:], bnsc_t[:], AluOp.mult)
                nc.vector.tensor_tensor(e_bn[:], e_bn[:], bnsh_t[:], AluOp.add)
                scr3 = pool.tile([BL, E], f32, name="scr3", tag="fin", bufs=1)
                nrm_sb = pool.tile([BL, 1], f32, name="nrm_sb", tag="nrm")
                nc.scalar.activation(
                    scr3[:], e_bn[:], mybir.ActivationFunctionType.Square,
                    accum_out=nrm_sb[:])
                nrms_sb = pool.tile([BL, 1], f32, name="nrms_sb", tag="nrms")
                nc.scalar.sqrt(nrms_sb[:], nrm_sb[:])
                rs_sb = pool.tile([BL, 1], f32, name="rs_sb", tag="nrmr")
                nc.vector.reciprocal(rs_sb[:], nrms_sb[:])
                e_fin = pool.tile([BL, E], f32, name="e_fin", tag="fin3", bufs=1)
                nc.vector.tensor_scalar_mul(e_fin[:], e_bn[:], rs_sb[:])
                nc.sync.dma_start(out[:], e_fin[:])

    _split_excess_waits(nc)
    return nc


def host_inputs(feat, W_proj, b_proj, bn_gamma, bn_beta, bn_mean, bn_var):
    """Build the 8 per-core input maps."""
    feat = np.ascontiguousarray(np.asarray(feat, dtype=np.float32))
    W_proj = np.asarray(W_proj, dtype=np.float32)
    featT = feat.reshape(B, C, M).transpose(0, 2, 1)          # [64, 196, 256]
    bnscale = (np.asarray(bn_gamma) /
               np.sqrt(np.asarray(bn_var) + BN_EPS)).astype(np.float32)
    bnshift = ((np.asarray(b_proj) - np.asarray(bn_mean)) * bnscale
               + np.asarray(bn_beta)).astype(np.float32)
    bnsc_rep = np.ascontiguousarray(np.broadcast_to(bnscale, (BL, E)))
    bnsh_rep = np.ascontiguousarray(np.broadcast_to(bnshift, (BL, E)))

    onesc = np.ones((128, 1), np.float32)
    onesr = np.ones((1, 128), np.float32)
    identS = np.zeros((128, 2 * C), np.float32)
    identS[:, 0:128] = np.eye(128, dtype=np.float32)          # chunk 0, j=p
    identS[:, C + 128:C + 256] = np.eye(128, dtype=np.float32)  # chunk 1, j=128+p

    in_maps = []
    for i in range(N_CORES):
        in_maps.append({
            "featT": np.ascontiguousarray(featT[i * BL:(i + 1) * BL]),
            "wT": np.ascontiguousarray(W_proj[:, KL * i:KL * (i + 1)].T),
            "onesc": onesc, "onesr": onesr, "identS": identS,
            "bnsc": bnsc_rep, "bnsh": bnsh_rep,
        })
    return in_maps


def kernel(feat, W_proj, b_proj, bn_gamma, bn_beta, bn_mean, bn_var):
    if "nc" not in _cache:
        _cache["nc"] = _build()
    nc = _cache["nc"]
    in_maps = host_inputs(feat, W_proj, b_proj, bn_gamma, bn_beta,
                          bn_mean, bn_var)
    last_err = None
    for _attempt in range(4):
        try:
            res = run_bass_kernel_spmd(nc, in_maps,
                                       core_ids=list(range(N_CORES)))
            break
        except Exception as e:  # transient NRT_EXEC_UNIT_UNRECOVERABLE flakes
            last_err = e
            import time as _time
            _time.sleep(2.0)
    else:
        raise last_err
    return np.concatenate([res.results[i]["out"] for i in range(N_CORES)],
                          axis=0)


# revision 14
# speedup vs baseline: 1.2619x; 1.2619x over previous
"""MPN-COV pooling + projection kernel for 8 Trainium2 NeuronCores.

Problem: nn_PillTeacher_48661979464182
  feat [64, 256, 14, 14] -> per-sample covariance + 3 Newton-Schulz sqrt
  iterations -> L2-normalize -> project with W_proj [512, 65536] -> BN -> L2.

Sharding:
  - Pooling phase: pure data parallel, 8 samples per core.
  - Projection: k-shard of W_proj (each core holds an 8192-wide slice of the
    contraction dim). AllToAll exchanges the normalized pooled matrices so
    every core gets its k-slice of all 64 samples; partial embeddings are
    summed with ReduceScatter back to the owning core of each sample.

Key tricks:
  - Every matrix in the Newton-Schulz iteration is a polynomial of the
    (symmetric) covariance -> symmetric -> matmul lhsT operands read the
    row-major tiles directly (no transposes on device; feat pre-transposed
    on host).
  - The final L2 normalization is invariant to any positive per-sample
    scale, so 1/M, 1/trace, sqrt(trY) and the 0.5 of the last NS Y-update
    all drop out.
  - fp32r (4x-rate fp32 matmul mode) for all matmuls.
  - BN + bias folded into a host-computed scale/shift.

Workarounds for this walrus build:
  - <=1 semaphore wait per instruction (_split_excess_waits post-pass).
  - no matmul with rhs free size 1 (scalar reductions go through [1, 256]
    row-sums + a free-axis reduce; scalar broadcasts use [1, 2] operands).
  - no tensor_tensor_reduce (mask-mult + tensor_reduce / activation instead).
"""
import sys
import numpy as np

sys.path.insert(0, "/opt/trn_rl_repo")

import concourse.bass as bass
import concourse.mybir as mybir
import concourse.tile as tile
import bass_rust
from concourse.bass_utils import run_bass_kernel_spmd

dt = mybir.dt

N_CORES = 8
B, C, H, W_SP = 64, 256, 14, 14
M = H * W_SP           # 196
E = 512
K = C * C              # 65536
BL = B // N_CORES      # 8 samples per core
KL = K // N_CORES      # 8192 contraction slice per core
BN_EPS = 1e-5

_cache = {}


def _split_excess_waits(nc, max_waits=1):
    """walrus in this env rejects >1 semaphore wait per instruction; hoist
    excess waits onto preceding NoOps on the same engine."""
    for fn in nc.m.functions:
        for bb in fn.blocks:
            new_insts = []
            for inst in bb.instructions:
                si = inst.sync_info
                if si is not None and si.on_wait and len(si.on_wait) > max_waits:
                    waits = list(si.on_wait)
                    chunks = [waits[i:i + max_waits]
                              for i in range(0, len(waits), max_waits)]
                    for chunk in chunks[:-1]:
                        nop = mybir.InstNoOp(
                            name=nc.get_next_instruction_name(), ins=[], outs=[],
                            engine=inst.engine)
                        nop.sync_info = bass_rust.SyncInfo(on_wait=chunk,
                                                           on_update=[])
                        new_insts.append(nop)
                    si.on_wait = chunks[-1]
                new_insts.append(inst)
            bb.instructions = new_insts


def _build(stage=5):
    """stage: 1=Y0 dump, 2=F dump, 3=a2a_out dump, 4=emb partial dump,
    5=full kernel.

    All 256x256 matrices use a stacked-pair layout: S[p, r*256 + j] =
    X[128*r + p, j] -- one [128, 512] tile per matrix, so every elementwise
    op is a single instruction and every PSUM product fills one full bank."""
    f32, f32r = dt.float32, dt.float32r
    nc = bass.Bass("TRN2", target_bir_lowering=False, debug=False,
                   num_devices=N_CORES)

    featT = nc.dram_tensor("featT", [BL, M, C], f32r, kind="ExternalInput")
    onesc = nc.dram_tensor("onesc", [128, 1], f32r, kind="ExternalInput")
    onesr = nc.dram_tensor("onesr", [1, 128], f32r, kind="ExternalInput")
    ident = nc.dram_tensor("identS", [128, 2 * C], f32, kind="ExternalInput")
    if stage >= 4:
        wT = nc.dram_tensor("wT", [KL, E], f32r, kind="ExternalInput")
    if stage >= 5:
        bnsc = nc.dram_tensor("bnsc", [BL, E], f32, kind="ExternalInput")
        bnsh = nc.dram_tensor("bnsh", [BL, E], f32, kind="ExternalInput")
        out = nc.dram_tensor("out", [BL, E], f32, kind="ExternalOutput")
    elif stage <= 2:
        out = nc.dram_tensor("dbg", [2, 128, C], f32, kind="ExternalOutput")
    elif stage == 3:
        out = nc.dram_tensor("dbg", [128, 4096], f32, kind="ExternalOutput")
    else:
        out = nc.dram_tensor("dbg", [64, E], f32, kind="ExternalOutput")

    rg = [list(range(N_CORES))]
    AluOp = mybir.AluOpType
    NCH = KL // 128        # 64 k-chunks for the projection

    lp = nc.allow_low_precision(reason="f32r intermediates carry fp32 bits")
    lp.__enter__()
    with tile.TileContext(nc) as tc:
        with (
            tc.tile_pool(name="consts", bufs=1) as cpool,
            tc.tile_pool(name="wbuf", bufs=1) as wpool,
            tc.tile_pool(name="big", bufs=1) as bigpool,
            tc.tile_pool(name="work", bufs=3) as pool,
            tc.tile_pool(name="mats", bufs=2) as mats,
            tc.tile_pool(name="pss", bufs=2, space="PSUM") as pss,
            tc.tile_pool(name="psg", bufs=3, space="PSUM") as psg,
            tc.tile_pool(name="pse", bufs=1, space="PSUM") as pse,
            tc.tile_pool(name="dram", bufs=1, space="DRAM") as dram,
        ):
            # ---------- constants ----------
            ones_t = cpool.tile([128, 1], f32r, name="ones_t")
            nc.sync.dma_start(ones_t[:], onesc[:])
            onesr_t = cpool.tile([1, 128], f32r, name="onesr_t")
            nc.sync.dma_start(onesr_t[:], onesr[:])
            identS_t = cpool.tile([128, 2 * C], f32, name="identS_t")
            threeIS_t = cpool.tile([128, 2 * C], f32, name="threeIS_t")
            nc.sync.dma_start(identS_t[:], ident[:])
            nc.scalar.mul(threeIS_t[:], identS_t[:], 3.0)
            if stage >= 5:
                bnsc_t = cpool.tile([BL, E], f32, name="bnsc_t")
                bnsh_t = cpool.tile([BL, E], f32, name="bnsh_t")
                nc.sync.dma_start(bnsc_t[:], bnsc[:])
                nc.sync.dma_start(bnsh_t[:], bnsh[:])

            # ---------- DRAM staging for collectives ----------
            if stage >= 3:
                # a2a flat layout: [j(8), h(2), p(128), b_l(8), i0(32)]
                a2a_in = dram.tile([128, 4096], f32r, name="a2a_in")
                a2a_out = dram.tile([128, 4096], f32r, name="a2a_out")
                a2a_in_v = a2a_in.flatten().rearrange(
                    "(j h p b i) -> h b p j i", j=8, h=2, p=128, b=BL, i=32)
            if stage >= 5:
                rs_in = dram.tile([B, E], f32, name="rs_in")
                rs_out = dram.tile([BL, E], f32, name="rs_out")

            def mmp(outS, AS, BS):
                """outS = A @ B for symmetric A, all in stacked-pair layout."""
                for r in range(2):
                    for kc in range(2):
                        nc.tensor.matmul(
                            outS[:, C * r:C * (r + 1)],
                            AS[:, C * kc + 128 * r:C * kc + 128 * r + 128],
                            BS[:, C * kc:C * (kc + 1)],
                            start=(kc == 0), stop=(kc == 1))

            def scalar_bcast(val_sb, tag):
                """[1,1] f32r scalar -> [128,1] f32 SBUF (via N=2 matmul)."""
                v2 = pool.tile([1, 2], f32r, name=f"v2{tag}", tag=f"v2{tag}")
                nc.vector.tensor_copy(v2[:, 0:1], val_sb[:])
                nc.vector.tensor_copy(v2[:, 1:2], val_sb[:])
                b_ps = pss.tile([128, 2], f32, name=f"bps{tag}", tag="sm2")
                nc.tensor.matmul(b_ps[:], onesr_t[:], v2[:],
                                 start=True, stop=True)
                b_sb = pool.tile([128, 1], f32, name=f"bsb{tag}", tag=f"bsb{tag}")
                nc.vector.tensor_copy(b_sb[:], b_ps[:, 0:1])
                return b_sb

            # ---------- pooling phase: BL samples ----------
            nsamp = 1 if stage <= 2 else BL
            # preload every sample's feat tiles first so the small feat DMAs
            # are not stuck behind the 16.8MB W prefetch in the DMA queues
            Bts = []
            for b in range(nsamp):
                B0 = pool.tile([128, C], f32r, name=f"B0_{b}", tag="B0",
                               bufs=nsamp)
                B1 = pool.tile([M - 128, C], f32r, name=f"B1_{b}", tag="B1",
                               bufs=nsamp)
                nc.sync.dma_start(B0[:], featT[b, 0:128, :])
                nc.sync.dma_start(B1[:], featT[b, 128:M, :])
                Bts.append((B0, B1))

            # ---------- W prefetch (issued after feat loads) ----------
            if stage >= 4:
                Wq = wpool.tile([128, NCH * E], f32r, name="Wq")
                wT_v = wT.rearrange("(g p) e -> p g e", p=128)   # [128,64,512]
                for g in range(16):
                    nc.sync.dma_start(
                        Wq[:, g * 4 * E:(g + 1) * 4 * E]
                            .rearrange("p (c e) -> p c e", c=4),
                        wT_v[:, 4 * g:4 * (g + 1), :])

            for b in range(nsamp):
                B0, B1 = Bts[b]

                # column sums -> [1, 256]
                srow_ps = pss.tile([1, 2 * C], f32, name="srow", tag="sm1")
                nc.tensor.matmul(srow_ps[:, 0:C], ones_t[0:128, :], B0[:],
                                 start=True, stop=False)
                nc.tensor.matmul(srow_ps[:, 0:C], ones_t[0:M - 128, :], B1[:],
                                 start=False, stop=True)
                s_sb = pool.tile([1, C], f32r, name="s_sb", tag="s_sb", bufs=1)
                t_sb = pool.tile([1, C], f32r, name="t_sb", tag="t_sb", bufs=1)
                nc.scalar.copy(s_sb[:], srow_ps[:, 0:C])
                nc.scalar.mul(t_sb[:], srow_ps[:, 0:C], -1.0 / M)

                # G = A^T A - M xbar xbar^T  (stacked PSUM [128, 512])
                GS = psg.tile([128, 2 * C], f32, name="GS", tag="Yp")
                for r in range(2):
                    nc.tensor.matmul(GS[:, C * r:C * (r + 1)],
                                     B0[:, 128 * r:128 * (r + 1)], B0[:],
                                     start=True, stop=False)
                    nc.tensor.matmul(GS[:, C * r:C * (r + 1)],
                                     B1[:, 128 * r:128 * (r + 1)], B1[:],
                                     start=False, stop=False)
                    nc.tensor.matmul(GS[:, C * r:C * (r + 1)],
                                     t_sb[:, 128 * r:128 * (r + 1)], s_sb[:],
                                     start=False, stop=True)

                # trace: diag mask -> column sums -> free-axis reduce
                scrS = pool.tile([128, 2 * C], f32r, name="scrS", tag="scr", bufs=2)
                nc.vector.tensor_tensor(scrS[:], GS[:], identS_t[:],
                                        AluOp.mult)
                trrow_ps = pss.tile([1, 2 * C], f32, name="trrow", tag="sm1")
                nc.tensor.matmul(trrow_ps[:], ones_t[0:128, :], scrS[:],
                                 start=True, stop=True)
                tr_sb = pool.tile([1, 1], f32, name="tr_sb", tag="tr_sb")
                nc.vector.tensor_reduce(out=tr_sb[:], in_=trrow_ps[:],
                                        axis=mybir.AxisListType.X,
                                        op=AluOp.add)
                inv_sb = pool.tile([1, 1], f32r, name="inv_sb", tag="inv")
                nc.vector.reciprocal(inv_sb[:], tr_sb[:])
                invb = scalar_bcast(inv_sb, "i")

                # Y0 = G / trG
                Y0S = mats.tile([128, 2 * C], f32r, name="Y0S", tag="Y0")
                nc.vector.tensor_scalar_mul(Y0S[:], GS[:], invb[:])

                if stage == 1:
                    for r in range(2):
                        nc.sync.dma_start(out[r, :, :],
                                          Y0S[:, C * r:C * (r + 1)].bitcast(f32))
                    break

                # ---- NS iter 1 (Z0=I): T1 = 3I - Y0; Y1 = .5 Y0 T1; Z1 = .5 T1
                T1S = mats.tile([128, 2 * C], f32r, name="T1S", tag="T")
                nc.vector.scalar_tensor_tensor(
                    out=T1S[:], in0=Y0S[:], scalar=-1.0, in1=threeIS_t[:],
                    op0=AluOp.mult, op1=AluOp.add)
                YpS = psg.tile([128, 2 * C], f32, name="YpS", tag="Yp")
                mmp(YpS, Y0S, T1S)
                Y1S = mats.tile([128, 2 * C], f32r, name="Y1S", tag="Y1")
                Z1S = mats.tile([128, 2 * C], f32r, name="Z1S", tag="Z")
                nc.scalar.mul(Y1S[:], YpS[:], 0.5)
                nc.scalar.mul(Z1S[:], T1S[:], 0.5)

                # ---- NS iter 2
                PpS = psg.tile([128, 2 * C], f32, name="PpS", tag="Yp")
                mmp(PpS, Z1S, Y1S)
                T2S = mats.tile([128, 2 * C], f32r, name="T2S", tag="T")
                nc.vector.scalar_tensor_tensor(
                    out=T2S[:], in0=PpS[:], scalar=-1.0, in1=threeIS_t[:],
                    op0=AluOp.mult, op1=AluOp.add)
                Yp2S = psg.tile([128, 2 * C], f32, name="Yp2S", tag="Yp")
                mmp(Yp2S, Y1S, T2S)
                Y2S = mats.tile([128, 2 * C], f32r, name="Y2S", tag="Y2")
                nc.scalar.mul(Y2S[:], Yp2S[:], 0.5)
                ZpS = psg.tile([128, 2 * C], f32, name="ZpS", tag="Yp")
                mmp(ZpS, T2S, Z1S)
                Z2S = mats.tile([128, 2 * C], f32r, name="Z2S", tag="Z")
                nc.scalar.mul(Z2S[:], ZpS[:], 0.5)

                # ---- NS iter 3 (Z dead): Y3 = Y2 (3I - Z2 Y2), unscaled
                Pp3S = psg.tile([128, 2 * C], f32, name="Pp3S", tag="Yp")
                mmp(Pp3S, Z2S, Y2S)
                T3S = mats.tile([128, 2 * C], f32r, name="T3S", tag="T")
                nc.vector.scalar_tensor_tensor(
                    out=T3S[:], in0=Pp3S[:], scalar=-1.0, in1=threeIS_t[:],
                    op0=AluOp.mult, op1=AluOp.add)
                Y3pS = psg.tile([128, 2 * C], f32, name="Y3pS", tag="Yp")
                mmp(Y3pS, Y2S, T3S)

                # ---- flat-normalize: F = Y3 / ||Y3||_F
                sqS = pool.tile([128, 2 * C], f32r, name="sqS", tag="scr", bufs=2)
                nc.scalar.activation(sqS[:], Y3pS[:],
                                     mybir.ActivationFunctionType.Square)
                ssqrow_ps = pss.tile([1, 2 * C], f32, name="ssqrow", tag="sm1")
                nc.tensor.matmul(ssqrow_ps[:], ones_t[0:128, :], sqS[:],
                                 start=True, stop=True)
                ssq_sb = pool.tile([1, 1], f32, name="ssq_sb", tag="tr_sb")
                nc.vector.tensor_reduce(out=ssq_sb[:], in_=ssqrow_ps[:],
                                        axis=mybir.AxisListType.X,
                                        op=AluOp.add)
                sqr_sb = pool.tile([1, 1], f32, name="sqr_sb", tag="sqr")
                nc.scalar.sqrt(sqr_sb[:], ssq_sb[:])
                rsq_sb = pool.tile([1, 1], f32r, name="rsq_sb", tag="inv")
                nc.vector.reciprocal(rsq_sb[:], sqr_sb[:])
                rsqb = scalar_bcast(rsq_sb, "r")
                FS = mats.tile([128, 2 * C], f32r, name="FS", tag="F")
                nc.vector.tensor_scalar_mul(FS[:], Y3pS[:], rsqb[:])

                if stage == 2:
                    for r in range(2):
                        nc.sync.dma_start(out[r, :, :],
                                          FS[:, C * r:C * (r + 1)].bitcast(f32))
                    break

                # ---- staging: dest j gets F[:, 32j:32j+32] (symmetry!)
                for hh in range(2):
                    nc.sync.dma_start(
                        a2a_in_v[hh, b],                          # [p, j, i]
                        FS[:, C * hh:C * (hh + 1)]
                            .rearrange("p (j i) -> p j i", j=8, i=32))

            # ---------- AllToAll ----------
            if stage >= 3:
                nc.gpsimd.collective_compute(
                    "AllToAll", AluOp.bypass, replica_groups=rg,
                    ins=[a2a_in.opt()], outs=[a2a_out.opt()])

            if stage == 3:
                tmp = bigpool.tile([128, 4096], f32, name="tmp")
                nc.sync.dma_start(tmp[:], a2a_out[:].bitcast(f32))
                nc.sync.dma_start(out[:], tmp[:])

            if stage >= 4:
                # ------- consumer: BIG [128, 4096], free = [h, s, b, i] -------
                BIG = bigpool.tile([128, 2 * 8 * BL * 32], f32r, name="BIG")
                a2a_out_v = a2a_out.flatten().rearrange(
                    "(s h p b i) -> h s p b i", s=8, h=2, p=128, b=BL, i=32)
                BIG_v = BIG[:].rearrange("p (h s b i) -> h s p b i",
                                         h=2, s=8, b=BL, i=32)
                for hh in range(2):
                    for s in range(8):
                        nc.sync.dma_start(BIG_v[hh, s], a2a_out_v[hh, s])

                # ------- projection: EMB[64, 512] -------
                EMB = pse.tile([64, E], f32, name="EMB")
                BIG_k = BIG[:].rearrange("p (h sb i) -> h i p sb",
                                         h=2, sb=64, i=32)
                for c in range(NCH):
                    i_local, hh = c // 2, c % 2
                    nc.tensor.matmul(
                        EMB[:], BIG_k[hh, i_local], Wq[:, E * c:E * (c + 1)],
                        start=(c == 0), stop=(c == NCH - 1))

                emb_sb = pool.tile([64, E], f32, name="emb_sb", tag="emb", bufs=1)
                nc.vector.tensor_copy(emb_sb[:], EMB[:])
                if stage == 4:
                    nc.sync.dma_start(out[:], emb_sb[:])

            if stage >= 5:
                nc.sync.dma_start(rs_in[:], emb_sb[:])

                # ------- ReduceScatter: [64, E] -> [8, E] -------
                nc.gpsimd.collective_compute(
                    "ReduceScatter", AluOp.add, replica_groups=rg,
                    ins=[rs_in.opt()], outs=[rs_out.opt()])

                # ------- BN fold + final L2 normalize -------
                e_sb = pool.tile([BL, E], f32, name="e_sb", tag="fin", bufs=1)
                nc.sync.dma_start(e_sb[:], rs_out[:])
                e_bn = pool.tile([BL, E], f32, name="e_bn", tag="fin2", bufs=1)
                nc.vector.tensor_tensor(e_bn[:], e_sb[:], bnsc_t[:], AluOp.mult)
                nc.vector.tensor_tensor(e_bn[:], e_bn[:], bnsh_t[:], AluOp.add)
                scr3 = pool.tile([BL, E], f32, name="scr3", tag="fin", bufs=1)
                nrm_sb = pool.tile([BL, 1], f32, name="nrm_sb", tag="nrm")
                nc.scalar.activation(
                    scr3[:], e_bn[:], mybir.ActivationFunctionType.Square,
                    accum_out=nrm_sb[:])
                nrms_sb = pool.tile([BL, 1], f32, name="nrms_sb", tag="nrms")
                nc.scalar.sqrt(nrms_sb[:], nrm_sb[:])
                rs_sb = pool.tile([BL, 1], f32, name="rs_sb", tag="nrmr")
                nc.vector.reciprocal(rs_sb[:], nrms_sb[:])
                e_fin = pool.tile([BL, E], f32, name="e_fin", tag="fin3", bufs=1)
                nc.vector.tensor_scalar_mul(e_fin[:], e_bn[:], rs_sb[:])
                nc.sync.dma_start(out[:], e_fin[:])

    _split_excess_waits(nc)
    return nc


def host_inputs(feat, W_proj, b_proj, bn_gamma, bn_beta, bn_mean, bn_var):
    """Build the 8 per-core input maps."""
    feat = np.ascontiguousarray(np.asarray(feat, dtype=np.float32))
    W_proj = np.asarray(W_proj, dtype=np.float32)
    featT = feat.reshape(B, C, M).transpose(0, 2, 1)          # [64, 196, 256]
    bnscale = (np.asarray(bn_gamma) /
               np.sqrt(np.asarray(bn_var) + BN_EPS)).astype(np.float32)
    bnshift = ((np.asarray(b_proj) - np.asarray(bn_mean)) * bnscale
               + np.asarray(bn_beta)).astype(np.float32)
    bnsc_rep = np.ascontiguousarray(np.broadcast_to(bnscale, (BL, E)))
    bnsh_rep = np.ascontiguousarray(np.broadcast_to(bnshift, (BL, E)))

    onesc = np.ones((128, 1), np.float32)
    onesr = np.ones((1, 128), np.float32)
    identS = np.zeros((128, 2 * C), np.float32)
    identS[:, 0:128] = np.eye(128, dtype=np.float32)          # chunk 0, j=p
    identS[:, C + 128:C + 256] = np.eye(128, dtype=np.float32)  # chunk 1, j=128+p

    in_maps = []
    for i in range(N_CORES):
        in_maps.append({
            "featT": np.ascontiguousarray(featT[i * BL:(i + 1) * BL]),
            "wT": np.ascontiguousarray(W_proj[:, KL * i:KL * (i + 1)].T),
            "onesc": onesc, "onesr": onesr, "identS": identS,
            "bnsc": bnsc_rep, "bnsh": bnsh_rep,
        })
    return in_maps


def kernel(feat, W_proj, b_proj, bn_gamma, bn_beta, bn_mean, bn_var):
    if "nc" not in _cache:
        _cache["nc"] = _build()
    nc = _cache["nc"]
    in_maps = host_inputs(feat, W_proj, b_proj, bn_gamma, bn_beta,
                          bn_mean, bn_var)
    last_err = None
    for _attempt in range(4):
        try:
            res = run_bass_kernel_spmd(nc, in_maps,
                                       core_ids=list(range(N_CORES)))
            break
        except Exception as e:  # transient NRT_EXEC_UNIT_UNRECOVERABLE flakes
            last_err = e
            import time as _time
            _time.sleep(2.0)
    else:
        raise last_err
    return np.concatenate([res.results[i]["out"] for i in range(N_CORES)],
                          axis=0)


# revision 15
# speedup vs baseline: 1.4772x; 1.1706x over previous
"""MPN-COV pooling + projection kernel for 8 Trainium2 NeuronCores.

Problem: nn_PillTeacher_48661979464182
  feat [64, 256, 14, 14] -> per-sample covariance + 3 Newton-Schulz sqrt
  iterations -> L2-normalize -> project with W_proj [512, 65536] -> BN -> L2.

Sharding:
  - Pooling phase: pure data parallel, 8 samples per core.
  - Projection: k-shard of W_proj (each core holds an 8192-wide slice of the
    contraction dim). AllToAll exchanges the normalized pooled matrices so
    every core gets its k-slice of all 64 samples; partial embeddings are
    summed with ReduceScatter back to the owning core of each sample.

Key tricks:
  - Every matrix in the Newton-Schulz iteration is a polynomial of the
    (symmetric) covariance -> symmetric -> matmul lhsT operands read the
    row-major tiles directly (no transposes on device; feat pre-transposed
    on host).
  - The final L2 normalization is invariant to any positive per-sample
    scale, so 1/M, 1/trace, sqrt(trY) and the 0.5 of the last NS Y-update
    all drop out.
  - fp32r (4x-rate fp32 matmul mode) for all matmuls.
  - BN + bias folded into a host-computed scale/shift.

Workarounds for this walrus build:
  - <=1 semaphore wait per instruction (_split_excess_waits post-pass).
  - no matmul with rhs free size 1 (scalar reductions go through [1, 256]
    row-sums + a free-axis reduce; scalar broadcasts use [1, 2] operands).
  - no tensor_tensor_reduce (mask-mult + tensor_reduce / activation instead).
"""
import sys
import numpy as np

sys.path.insert(0, "/opt/trn_rl_repo")

import concourse.bass as bass
import concourse.mybir as mybir
import concourse.tile as tile
import bass_rust
from concourse.bass_utils import run_bass_kernel_spmd

dt = mybir.dt

N_CORES = 8
B, C, H, W_SP = 64, 256, 14, 14
M = H * W_SP           # 196
E = 512
K = C * C              # 65536
BL = B // N_CORES      # 8 samples per core
KL = K // N_CORES      # 8192 contraction slice per core
BN_EPS = 1e-5

_cache = {}


def _split_excess_waits(nc, max_waits=1):
    """walrus in this env rejects >1 semaphore wait per instruction; hoist
    excess waits onto preceding NoOps on the same engine."""
    for fn in nc.m.functions:
        for bb in fn.blocks:
            new_insts = []
            for inst in bb.instructions:
                si = inst.sync_info
                if si is not None and si.on_wait and len(si.on_wait) > max_waits:
                    waits = list(si.on_wait)
                    chunks = [waits[i:i + max_waits]
                              for i in range(0, len(waits), max_waits)]
                    for chunk in chunks[:-1]:
                        nop = mybir.InstNoOp(
                            name=nc.get_next_instruction_name(), ins=[], outs=[],
                            engine=inst.engine)
                        nop.sync_info = bass_rust.SyncInfo(on_wait=chunk,
                                                           on_update=[])
                        new_insts.append(nop)
                    si.on_wait = chunks[-1]
                new_insts.append(inst)
            bb.instructions = new_insts


def _build(stage=5):
    """stage: 1=Y0 dump, 2=F dump, 3=a2a_out dump, 4=emb partial dump,
    5=full kernel.

    All 256x256 matrices use a stacked-pair layout: S[p, r*256 + j] =
    X[128*r + p, j] -- one [128, 512] tile per matrix, so every elementwise
    op is a single instruction and every PSUM product fills one full bank."""
    f32, f32r = dt.float32, dt.float32r
    nc = bass.Bass("TRN2", target_bir_lowering=False, debug=False,
                   num_devices=N_CORES)

    featT = nc.dram_tensor("featT", [BL, M, C], f32r, kind="ExternalInput")
    onesc = nc.dram_tensor("onesc", [128, 1], f32r, kind="ExternalInput")
    onesr = nc.dram_tensor("onesr", [1, 128], f32r, kind="ExternalInput")
    ident = nc.dram_tensor("identS", [128, 2 * C], f32, kind="ExternalInput")
    if stage >= 4:
        wT = nc.dram_tensor("wT", [KL, E], f32r, kind="ExternalInput")
    if stage >= 5:
        bnsc = nc.dram_tensor("bnsc", [BL, E], f32, kind="ExternalInput")
        bnsh = nc.dram_tensor("bnsh", [BL, E], f32, kind="ExternalInput")
        out = nc.dram_tensor("out", [BL, E], f32, kind="ExternalOutput")
    elif stage <= 2:
        out = nc.dram_tensor("dbg", [2, 128, C], f32, kind="ExternalOutput")
    elif stage == 3:
        out = nc.dram_tensor("dbg", [128, 4096], f32, kind="ExternalOutput")
    else:
        out = nc.dram_tensor("dbg", [64, E], f32, kind="ExternalOutput")

    rg = [list(range(N_CORES))]
    AluOp = mybir.AluOpType
    NCH = KL // 128        # 64 k-chunks for the projection

    lp = nc.allow_low_precision(reason="f32r intermediates carry fp32 bits")
    lp.__enter__()
    with tile.TileContext(nc) as tc:
        with (
            tc.tile_pool(name="consts", bufs=1) as cpool,
            tc.tile_pool(name="wbuf", bufs=1) as wpool,
            tc.tile_pool(name="big", bufs=1) as bigpool,
            tc.tile_pool(name="work", bufs=3) as pool,
            tc.tile_pool(name="mats", bufs=2) as mats,
            tc.tile_pool(name="pss", bufs=2, space="PSUM") as pss,
            tc.tile_pool(name="psg", bufs=4, space="PSUM") as psg,
            tc.tile_pool(name="pse", bufs=1, space="PSUM") as pse,
            tc.tile_pool(name="dram", bufs=1, space="DRAM") as dram,
        ):
            # ---------- constants ----------
            ones_t = cpool.tile([128, 1], f32r, name="ones_t")
            nc.sync.dma_start(ones_t[:], onesc[:])
            onesr_t = cpool.tile([1, 128], f32r, name="onesr_t")
            nc.sync.dma_start(onesr_t[:], onesr[:])
            identS_t = cpool.tile([128, 2 * C], f32, name="identS_t")
            threeIS_t = cpool.tile([128, 2 * C], f32, name="threeIS_t")
            nc.sync.dma_start(identS_t[:], ident[:])
            nc.scalar.mul(threeIS_t[:], identS_t[:], 3.0)
            if stage >= 5:
                bnsc_t = cpool.tile([BL, E], f32, name="bnsc_t")
                bnsh_t = cpool.tile([BL, E], f32, name="bnsh_t")
                nc.sync.dma_start(bnsc_t[:], bnsc[:])
                nc.sync.dma_start(bnsh_t[:], bnsh[:])

            # ---------- DRAM staging for collectives ----------
            if stage >= 3:
                # a2a flat layout: [j(8), h(2), p(128), b_l(8), i0(32)]
                a2a_in = dram.tile([128, 4096], f32r, name="a2a_in")
                a2a_out = dram.tile([128, 4096], f32r, name="a2a_out")
                a2a_in_v = a2a_in.flatten().rearrange(
                    "(j h p b i) -> h b p j i", j=8, h=2, p=128, b=BL, i=32)
            if stage >= 5:
                rs_in = dram.tile([B, E], f32, name="rs_in")
                rs_out = dram.tile([BL, E], f32, name="rs_out")

            def mmp(outS, AS, BS):
                """outS = A @ B for symmetric A, all in stacked-pair layout."""
                for r in range(2):
                    for kc in range(2):
                        nc.tensor.matmul(
                            outS[:, C * r:C * (r + 1)],
                            AS[:, C * kc + 128 * r:C * kc + 128 * r + 128],
                            BS[:, C * kc:C * (kc + 1)],
                            start=(kc == 0), stop=(kc == 1))

            def scalar_bcast(val_sb, tag):
                """[1,1] f32r scalar -> [128,1] f32 SBUF (via N=2 matmul)."""
                v2 = pool.tile([1, 2], f32r, name=f"v2{tag}", tag=f"v2{tag}")
                nc.vector.tensor_copy(v2[:, 0:1], val_sb[:])
                nc.vector.tensor_copy(v2[:, 1:2], val_sb[:])
                b_ps = pss.tile([128, 2], f32, name=f"bps{tag}", tag="sm2", bufs=1)
                nc.tensor.matmul(b_ps[:], onesr_t[:], v2[:],
                                 start=True, stop=True)
                b_sb = pool.tile([128, 1], f32, name=f"bsb{tag}", tag=f"bsb{tag}")
                nc.vector.tensor_copy(b_sb[:], b_ps[:, 0:1])
                return b_sb

            # ---------- pooling phase: BL samples, stage-major in groups ----------
            nsamp = 1 if stage <= 2 else BL
            GD = min(4, nsamp)     # software-pipeline depth

            # preload every sample's feat tiles first (small DMAs ahead of
            # everything else in the queues)
            Bts = []
            for b in range(nsamp):
                B0 = pool.tile([128, C], f32r, name=f"B0_{b}", tag="B0",
                               bufs=nsamp)
                B1 = pool.tile([M - 128, C], f32r, name=f"B1_{b}", tag="B1",
                               bufs=nsamp)
                nc.sync.dma_start(B0[:], featT[b, 0:128, :])
                nc.sync.dma_start(B1[:], featT[b, 128:M, :])
                Bts.append((B0, B1))

            for g0 in range(0, nsamp, GD):
                gb = list(range(g0, min(g0 + GD, nsamp)))
                st = {b: {} for b in gb}

                # -- column sums
                for b in gb:
                    B0, B1 = Bts[b]
                    srow_ps = pss.tile([1, 2 * C], f32, name=f"srow{b}",
                                       tag="sm1")
                    nc.tensor.matmul(srow_ps[:, 0:C], ones_t[0:128, :], B0[:],
                                     start=True, stop=False)
                    nc.tensor.matmul(srow_ps[:, 0:C], ones_t[0:M - 128, :],
                                     B1[:], start=False, stop=True)
                    st[b]["srow"] = srow_ps
                for b in gb:
                    s_sb = pool.tile([1, C], f32r, name=f"s_sb{b}", tag="s_sb",
                                     bufs=GD)
                    t_sb = pool.tile([1, C], f32r, name=f"t_sb{b}", tag="t_sb",
                                     bufs=GD)
                    nc.scalar.copy(s_sb[:], st[b]["srow"][:, 0:C])
                    nc.scalar.mul(t_sb[:], st[b]["srow"][:, 0:C], -1.0 / M)
                    st[b]["s"], st[b]["t"] = s_sb, t_sb

                # -- G = A^T A - M xbar xbar^T
                for b in gb:
                    B0, B1 = Bts[b]
                    GS = psg.tile([128, 2 * C], f32, name=f"GS{b}", tag="Yp")
                    for r in range(2):
                        nc.tensor.matmul(GS[:, C * r:C * (r + 1)],
                                         B0[:, 128 * r:128 * (r + 1)], B0[:],
                                         start=True, stop=False)
                        nc.tensor.matmul(GS[:, C * r:C * (r + 1)],
                                         B1[:, 128 * r:128 * (r + 1)], B1[:],
                                         start=False, stop=False)
                        nc.tensor.matmul(GS[:, C * r:C * (r + 1)],
                                         st[b]["t"][:, 128 * r:128 * (r + 1)],
                                         st[b]["s"][:], start=False, stop=True)
                    st[b]["G"] = GS

                # -- trace -> 1/trG broadcast
                for b in gb:
                    scrS = pool.tile([128, 2 * C], f32r, name=f"scrS{b}",
                                     tag="scr", bufs=GD)
                    nc.vector.tensor_tensor(scrS[:], st[b]["G"][:],
                                            identS_t[:], AluOp.mult)
                    st[b]["scr"] = scrS
                for b in gb:
                    trrow_ps = pss.tile([1, 2 * C], f32, name=f"trrow{b}",
                                        tag="sm1")
                    nc.tensor.matmul(trrow_ps[:], ones_t[0:128, :],
                                     st[b]["scr"][:], start=True, stop=True)
                    st[b]["trrow"] = trrow_ps
                for b in gb:
                    tr_sb = pool.tile([1, 1], f32, name=f"tr_sb{b}",
                                      tag="tr_sb", bufs=GD)
                    nc.vector.tensor_reduce(out=tr_sb[:], in_=st[b]["trrow"][:],
                                            axis=mybir.AxisListType.X,
                                            op=AluOp.add)
                    inv_sb = pool.tile([1, 1], f32r, name=f"inv_sb{b}",
                                       tag="inv", bufs=GD)
                    nc.vector.reciprocal(inv_sb[:], tr_sb[:])
                    st[b]["invb"] = scalar_bcast(inv_sb, f"i{b}")

                # -- Y0 = G/trG ; T1 = 3I - Y0
                for b in gb:
                    Y0S = mats.tile([128, 2 * C], f32r, name=f"Y0S{b}",
                                    tag="Y0", bufs=GD)
                    nc.vector.tensor_scalar_mul(Y0S[:], st[b]["G"][:],
                                                st[b]["invb"][:])
                    st[b]["Y0"] = Y0S

                if stage == 1:
                    for r in range(2):
                        nc.sync.dma_start(
                            out[r, :, :],
                            st[gb[0]]["Y0"][:, C * r:C * (r + 1)].bitcast(f32))
                    break

                for b in gb:
                    T1S = mats.tile([128, 2 * C], f32r, name=f"T1S{b}",
                                    tag="T", bufs=GD)
                    nc.vector.scalar_tensor_tensor(
                        out=T1S[:], in0=st[b]["Y0"][:], scalar=-1.0,
                        in1=threeIS_t[:], op0=AluOp.mult, op1=AluOp.add)
                    st[b]["T1"] = T1S

                # -- iter1 products; Y1 = .5 Y0 T1 ; Z1 = .5 T1
                for b in gb:
                    YpS = psg.tile([128, 2 * C], f32, name=f"YpS{b}", tag="Yp")
                    mmp(YpS, st[b]["Y0"], st[b]["T1"])
                    st[b]["Yp"] = YpS
                for b in gb:
                    Y1S = mats.tile([128, 2 * C], f32r, name=f"Y1S{b}",
                                    tag="Y1", bufs=GD)
                    Z1S = mats.tile([128, 2 * C], f32r, name=f"Z1S{b}",
                                    tag="Z", bufs=GD)
                    nc.scalar.mul(Y1S[:], st[b]["Yp"][:], 0.5)
                    nc.scalar.mul(Z1S[:], st[b]["T1"][:], 0.5)
                    st[b]["Y1"], st[b]["Z1"] = Y1S, Z1S

                # -- iter2
                for b in gb:
                    PpS = psg.tile([128, 2 * C], f32, name=f"PpS{b}", tag="Yp")
                    mmp(PpS, st[b]["Z1"], st[b]["Y1"])
                    st[b]["Pp"] = PpS
                for b in gb:
                    T2S = mats.tile([128, 2 * C], f32r, name=f"T2S{b}",
                                    tag="T", bufs=GD)
                    nc.vector.scalar_tensor_tensor(
                        out=T2S[:], in0=st[b]["Pp"][:], scalar=-1.0,
                        in1=threeIS_t[:], op0=AluOp.mult, op1=AluOp.add)
                    st[b]["T2"] = T2S
                for b in gb:
                    Yp2S = psg.tile([128, 2 * C], f32, name=f"Yp2S{b}",
                                    tag="Yp")
                    mmp(Yp2S, st[b]["Y1"], st[b]["T2"])
                    st[b]["Yp2"] = Yp2S
                for b in gb:
                    Y2S = mats.tile([128, 2 * C], f32r, name=f"Y2S{b}",
                                    tag="Y2", bufs=GD)
                    nc.scalar.mul(Y2S[:], st[b]["Yp2"][:], 0.5)
                    st[b]["Y2"] = Y2S
                for b in gb:
                    ZpS = psg.tile([128, 2 * C], f32, name=f"ZpS{b}", tag="Yp")
                    mmp(ZpS, st[b]["T2"], st[b]["Z1"])
                    st[b]["Zp"] = ZpS
                for b in gb:
                    Z2S = mats.tile([128, 2 * C], f32r, name=f"Z2S{b}",
                                    tag="Z", bufs=GD)
                    nc.scalar.mul(Z2S[:], st[b]["Zp"][:], 0.5)
                    st[b]["Z2"] = Z2S

                # -- iter3 (Z dead)
                for b in gb:
                    Pp3S = psg.tile([128, 2 * C], f32, name=f"Pp3S{b}",
                                    tag="Yp")
                    mmp(Pp3S, st[b]["Z2"], st[b]["Y2"])
                    st[b]["Pp3"] = Pp3S
                for b in gb:
                    T3S = mats.tile([128, 2 * C], f32r, name=f"T3S{b}",
                                    tag="T", bufs=GD)
                    nc.vector.scalar_tensor_tensor(
                        out=T3S[:], in0=st[b]["Pp3"][:], scalar=-1.0,
                        in1=threeIS_t[:], op0=AluOp.mult, op1=AluOp.add)
                    st[b]["T3"] = T3S
                for b in gb:
                    Y3pS = psg.tile([128, 2 * C], f32, name=f"Y3pS{b}",
                                    tag="Yp")
                    mmp(Y3pS, st[b]["Y2"], st[b]["T3"])
                    st[b]["Y3p"] = Y3pS

                # -- flat-normalize + staging
                for b in gb:
                    sqS = pool.tile([128, 2 * C], f32r, name=f"sqS{b}",
                                    tag="scr", bufs=GD)
                    nc.scalar.activation(sqS[:], st[b]["Y3p"][:],
                                         mybir.ActivationFunctionType.Square)
                    st[b]["sq"] = sqS
                for b in gb:
                    ssqrow_ps = pss.tile([1, 2 * C], f32, name=f"ssqrow{b}",
                                         tag="sm1")
                    nc.tensor.matmul(ssqrow_ps[:], ones_t[0:128, :],
                                     st[b]["sq"][:], start=True, stop=True)
                    st[b]["ssqrow"] = ssqrow_ps
                for b in gb:
                    ssq_sb = pool.tile([1, 1], f32, name=f"ssq_sb{b}",
                                       tag="tr_sb", bufs=GD)
                    nc.vector.tensor_reduce(out=ssq_sb[:],
                                            in_=st[b]["ssqrow"][:],
                                            axis=mybir.AxisListType.X,
                                            op=AluOp.add)
                    sqr_sb = pool.tile([1, 1], f32, name=f"sqr_sb{b}",
                                       tag="sqr", bufs=GD)
                    nc.scalar.sqrt(sqr_sb[:], ssq_sb[:])
                    rsq_sb = pool.tile([1, 1], f32r, name=f"rsq_sb{b}",
                                       tag="inv", bufs=GD)
                    nc.vector.reciprocal(rsq_sb[:], sqr_sb[:])
                    st[b]["rsqb"] = scalar_bcast(rsq_sb, f"r{b}")
                for b in gb:
                    FS = mats.tile([128, 2 * C], f32r, name=f"FS{b}", tag="F",
                                   bufs=GD)
                    nc.vector.tensor_scalar_mul(FS[:], st[b]["Y3p"][:],
                                                st[b]["rsqb"][:])
                    st[b]["F"] = FS

                if stage == 2:
                    for r in range(2):
                        nc.sync.dma_start(
                            out[r, :, :],
                            st[gb[0]]["F"][:, C * r:C * (r + 1)].bitcast(f32))
                    break

                for b in gb:
                    for hh in range(2):
                        nc.sync.dma_start(
                            a2a_in_v[hh, b],                      # [p, j, i]
                            st[b]["F"][:, C * hh:C * (hh + 1)]
                                .rearrange("p (j i) -> p j i", j=8, i=32))

            # ---------- AllToAll ----------
            if stage >= 3:
                nc.gpsimd.collective_compute(
                    "AllToAll", AluOp.bypass, replica_groups=rg,
                    ins=[a2a_in.opt()], outs=[a2a_out.opt()])

            if stage == 3:
                tmp = bigpool.tile([128, 4096], f32, name="tmp")
                nc.sync.dma_start(tmp[:], a2a_out[:].bitcast(f32))
                nc.sync.dma_start(out[:], tmp[:])

            if stage >= 4:
                # ------- consumer: BIG [128, 4096], free = [h, s, b, i] -------
                BIG = bigpool.tile([128, 2 * 8 * BL * 32], f32r, name="BIG")
                a2a_out_v = a2a_out.flatten().rearrange(
                    "(s h p b i) -> h s p b i", s=8, h=2, p=128, b=BL, i=32)
                BIG_v = BIG[:].rearrange("p (h s b i) -> h s p b i",
                                         h=2, s=8, b=BL, i=32)
                for hh in range(2):
                    for s in range(8):
                        nc.sync.dma_start(BIG_v[hh, s], a2a_out_v[hh, s])

                # ------- projection: EMB[64, 512], W streamed -------
                EMB = pse.tile([64, E], f32, name="EMB")
                BIG_k = BIG[:].rearrange("p (h sb i) -> h i p sb",
                                         h=2, sb=64, i=32)
                wT_v = wT.rearrange("(c p) e -> c p e", p=128)  # [64,128,512]
                for c in range(NCH):
                    i_local, hh = c // 2, c % 2
                    wq = wpool.tile([128, E], f32r, name=f"wq{c}", tag="wq",
                                    bufs=24)
                    nc.sync.dma_start(wq[:], wT_v[c])
                    nc.tensor.matmul(
                        EMB[:], BIG_k[hh, i_local], wq[:],
                        start=(c == 0), stop=(c == NCH - 1))

                emb_sb = pool.tile([64, E], f32, name="emb_sb", tag="emb", bufs=1)
                nc.vector.tensor_copy(emb_sb[:], EMB[:])
                if stage == 4:
                    nc.sync.dma_start(out[:], emb_sb[:])

            if stage >= 5:
                nc.sync.dma_start(rs_in[:], emb_sb[:])

                # ------- ReduceScatter: [64, E] -> [8, E] -------
                nc.gpsimd.collective_compute(
                    "ReduceScatter", AluOp.add, replica_groups=rg,
                    ins=[rs_in.opt()], outs=[rs_out.opt()])

                # ------- BN fold + final L2 normalize -------
                e_sb = pool.tile([BL, E], f32, name="e_sb", tag="fin", bufs=1)
                nc.sync.dma_start(e_sb[:], rs_out[:])
                e_bn = pool.tile([BL, E], f32, name="e_bn", tag="fin2", bufs=1)
                nc.vector.tensor_tensor(e_bn[:], e_sb[:], bnsc_t[:], AluOp.mult)
                nc.vector.tensor_tensor(e_bn[:], e_bn[:], bnsh_t[:], AluOp.add)
                scr3 = pool.tile([BL, E], f32, name="scr3", tag="fin", bufs=1)
                nrm_sb = pool.tile([BL, 1], f32, name="nrm_sb", tag="nrm")
                nc.scalar.activation(
                    scr3[:], e_bn[:], mybir.ActivationFunctionType.Square,
                    accum_out=nrm_sb[:])
                nrms_sb = pool.tile([BL, 1], f32, name="nrms_sb", tag="nrms")
                nc.scalar.sqrt(nrms_sb[:], nrm_sb[:])
                rs_sb = pool.tile([BL, 1], f32, name="rs_sb", tag="nrmr")
                nc.vector.reciprocal(rs_sb[:], nrms_sb[:])
                e_fin = pool.tile([BL, E], f32, name="e_fin", tag="fin3", bufs=1)
                nc.vector.tensor_scalar_mul(e_fin[:], e_bn[:], rs_sb[:])
                nc.sync.dma_start(out[:], e_fin[:])

    _split_excess_waits(nc)
    return nc


def host_inputs(feat, W_proj, b_proj, bn_gamma, bn_beta, bn_mean, bn_var):
    """Build the 8 per-core input maps."""
    feat = np.ascontiguousarray(np.asarray(feat, dtype=np.float32))
    W_proj = np.asarray(W_proj, dtype=np.float32)
    featT = feat.reshape(B, C, M).transpose(0, 2, 1)          # [64, 196, 256]
    bnscale = (np.asarray(bn_gamma) /
               np.sqrt(np.asarray(bn_var) + BN_EPS)).astype(np.float32)
    bnshift = ((np.asarray(b_proj) - np.asarray(bn_mean)) * bnscale
               + np.asarray(bn_beta)).astype(np.float32)
    bnsc_rep = np.ascontiguousarray(np.broadcast_to(bnscale, (BL, E)))
    bnsh_rep = np.ascontiguousarray(np.broadcast_to(bnshift, (BL, E)))

    onesc = np.ones((128, 1), np.float32)
    onesr = np.ones((1, 128), np.float32)
    identS = np.zeros((128, 2 * C), np.float32)
    identS[:, 0:128] = np.eye(128, dtype=np.float32)          # chunk 0, j=p
    identS[:, C + 128:C + 256] = np.eye(128, dtype=np.float32)  # chunk 1, j=128+p

    in_maps = []
    for i in range(N_CORES):
        in_maps.append({
            "featT": np.ascontiguousarray(featT[i * BL:(i + 1) * BL]),
            "wT": np.ascontiguousarray(W_proj[:, KL * i:KL * (i + 1)].T),
            "onesc": onesc, "onesr": onesr, "identS": identS,
            "bnsc": bnsc_rep, "bnsh": bnsh_rep,
        })
    return in_maps


def kernel(feat, W_proj, b_proj, bn_gamma, bn_beta, bn_mean, bn_var):
    if "nc" not in _cache:
        _cache["nc"] = _build()
    nc = _cache["nc"]
    in_maps = host_inputs(feat, W_proj, b_proj, bn_gamma, bn_beta,
                          bn_mean, bn_var)
    last_err = None
    for _attempt in range(4):
        try:
            res = run_bass_kernel_spmd(nc, in_maps,
                                       core_ids=list(range(N_CORES)))
            break
        except Exception as e:  # transient NRT_EXEC_UNIT_UNRECOVERABLE flakes
            last_err = e
            import time as _time
            _time.sleep(2.0)
    else:
        raise last_err
    return np.concatenate([res.results[i]["out"] for i in range(N_CORES)],
                          axis=0)


# revision 16
# speedup vs baseline: 1.5635x; 1.0584x over previous
"""MPN-COV pooling + projection kernel for 8 Trainium2 NeuronCores.

Problem: nn_PillTeacher_48661979464182
  feat [64, 256, 14, 14] -> per-sample covariance + 3 Newton-Schulz sqrt
  iterations -> L2-normalize -> project with W_proj [512, 65536] -> BN -> L2.

Sharding:
  - Pooling phase: pure data parallel, 8 samples per core.
  - Projection: k-shard of W_proj (each core holds an 8192-wide slice of the
    contraction dim). AllToAll exchanges the normalized pooled matrices so
    every core gets its k-slice of all 64 samples; partial embeddings are
    summed with ReduceScatter back to the owning core of each sample.

Key tricks:
  - Every matrix in the Newton-Schulz iteration is a polynomial of the
    (symmetric) covariance -> symmetric -> matmul lhsT operands read the
    row-major tiles directly (no transposes on device; feat pre-transposed
    on host).
  - The final L2 normalization is invariant to any positive per-sample
    scale, so 1/M, 1/trace, sqrt(trY) and the 0.5 of the last NS Y-update
    all drop out.
  - fp32r (4x-rate fp32 matmul mode) for all matmuls.
  - BN + bias folded into a host-computed scale/shift.

Workarounds for this walrus build:
  - <=1 semaphore wait per instruction (_split_excess_waits post-pass).
  - no matmul with rhs free size 1 (scalar reductions go through [1, 256]
    row-sums + a free-axis reduce; scalar broadcasts use [1, 2] operands).
  - no tensor_tensor_reduce (mask-mult + tensor_reduce / activation instead).
"""
import sys
import numpy as np

sys.path.insert(0, "/opt/trn_rl_repo")

import concourse.bass as bass
import concourse.mybir as mybir
import concourse.tile as tile
import bass_rust
from concourse.bass_utils import run_bass_kernel_spmd

dt = mybir.dt

N_CORES = 8
B, C, H, W_SP = 64, 256, 14, 14
M = H * W_SP           # 196
E = 512
K = C * C              # 65536
BL = B // N_CORES      # 8 samples per core
KL = K // N_CORES      # 8192 contraction slice per core
BN_EPS = 1e-5

_cache = {}


def _split_excess_waits(nc, max_waits=1):
    """walrus in this env rejects >1 semaphore wait per instruction; hoist
    excess waits onto preceding NoOps on the same engine."""
    for fn in nc.m.functions:
        for bb in fn.blocks:
            new_insts = []
            for inst in bb.instructions:
                si = inst.sync_info
                if si is not None and si.on_wait and len(si.on_wait) > max_waits:
                    waits = list(si.on_wait)
                    chunks = [waits[i:i + max_waits]
                              for i in range(0, len(waits), max_waits)]
                    for chunk in chunks[:-1]:
                        nop = mybir.InstNoOp(
                            name=nc.get_next_instruction_name(), ins=[], outs=[],
                            engine=inst.engine)
                        nop.sync_info = bass_rust.SyncInfo(on_wait=chunk,
                                                           on_update=[])
                        new_insts.append(nop)
                    si.on_wait = chunks[-1]
                new_insts.append(inst)
            bb.instructions = new_insts


def _build(stage=5):
    """stage: 1=Y0 dump, 2=F dump, 3=a2a_out dump, 4=emb partial dump,
    5=full kernel.

    All 256x256 matrices use a stacked-pair layout: S[p, r*256 + j] =
    X[128*r + p, j] -- one [128, 512] tile per matrix, so every elementwise
    op is a single instruction and every PSUM product fills one full bank."""
    f32, f32r = dt.float32, dt.float32r
    nc = bass.Bass("TRN2", target_bir_lowering=False, debug=False,
                   num_devices=N_CORES)

    featT = nc.dram_tensor("featT", [BL, M, C], f32r, kind="ExternalInput")
    onesc = nc.dram_tensor("onesc", [128, 1], f32r, kind="ExternalInput")
    onesr = nc.dram_tensor("onesr", [1, 128], f32r, kind="ExternalInput")
    ident = nc.dram_tensor("identS", [128, 2 * C], f32, kind="ExternalInput")
    if stage >= 4:
        wT = nc.dram_tensor("wT", [KL, E], f32r, kind="ExternalInput")
    if stage >= 5:
        bnsc = nc.dram_tensor("bnsc", [BL, E], f32, kind="ExternalInput")
        bnsh = nc.dram_tensor("bnsh", [BL, E], f32, kind="ExternalInput")
        out = nc.dram_tensor("out", [BL, E], f32, kind="ExternalOutput")
    elif stage <= 2:
        out = nc.dram_tensor("dbg", [2, 128, C], f32, kind="ExternalOutput")
    elif stage == 3:
        out = nc.dram_tensor("dbg", [128, 4096], f32, kind="ExternalOutput")
    else:
        out = nc.dram_tensor("dbg", [64, E], f32, kind="ExternalOutput")

    rg = [list(range(N_CORES))]
    AluOp = mybir.AluOpType
    NCH = KL // 128        # 64 k-chunks for the projection

    lp = nc.allow_low_precision(reason="f32r intermediates carry fp32 bits")
    lp.__enter__()
    with tile.TileContext(nc) as tc:
        with (
            tc.tile_pool(name="consts", bufs=1) as cpool,
            tc.tile_pool(name="wbuf", bufs=1) as wpool,
            tc.tile_pool(name="big", bufs=1) as bigpool,
            tc.tile_pool(name="work", bufs=3) as pool,
            tc.tile_pool(name="mats", bufs=2) as mats,
            tc.tile_pool(name="pss", bufs=2, space="PSUM") as pss,
            tc.tile_pool(name="psg", bufs=4, space="PSUM") as psg,
            tc.tile_pool(name="pse", bufs=1, space="PSUM") as pse,
            tc.tile_pool(name="dram", bufs=1, space="DRAM") as dram,
        ):
            # ---------- constants ----------
            ones_t = cpool.tile([128, 1], f32r, name="ones_t")
            nc.sync.dma_start(ones_t[:], onesc[:])
            onesr_t = cpool.tile([1, 128], f32r, name="onesr_t")
            nc.sync.dma_start(onesr_t[:], onesr[:])
            identS_t = cpool.tile([128, 2 * C], f32, name="identS_t")
            threeIS_t = cpool.tile([128, 2 * C], f32, name="threeIS_t")
            nc.sync.dma_start(identS_t[:], ident[:])
            nc.scalar.mul(threeIS_t[:], identS_t[:], 3.0)
            if stage >= 5:
                bnsc_t = cpool.tile([BL, E], f32, name="bnsc_t")
                bnsh_t = cpool.tile([BL, E], f32, name="bnsh_t")
                nc.sync.dma_start(bnsc_t[:], bnsc[:])
                nc.sync.dma_start(bnsh_t[:], bnsh[:])

            # ---------- DRAM staging for collectives ----------
            if stage >= 3:
                # a2a flat layout: [j(8), h(2), p(128), b_l(8), i0(32)]
                a2a_in = dram.tile([128, 4096], f32r, name="a2a_in")
                a2a_out = dram.tile([128, 4096], f32r, name="a2a_out")
                a2a_in_v = a2a_in.flatten().rearrange(
                    "(j h p b i) -> h b p j i", j=8, h=2, p=128, b=BL, i=32)
            if stage >= 5:
                rs_in = dram.tile([B, E], f32, name="rs_in")
                rs_out = dram.tile([BL, E], f32, name="rs_out")

            def mmp(outS, AS, BS):
                """outS = A @ B for symmetric A, all in stacked-pair layout."""
                for r in range(2):
                    for kc in range(2):
                        nc.tensor.matmul(
                            outS[:, C * r:C * (r + 1)],
                            AS[:, C * kc + 128 * r:C * kc + 128 * r + 128],
                            BS[:, C * kc:C * (kc + 1)],
                            start=(kc == 0), stop=(kc == 1))

            def scalar_bcast(val_sb, tag):
                """[1,1] f32r scalar -> [128,1] f32 SBUF (via N=2 matmul)."""
                v2 = pool.tile([1, 2], f32r, name=f"v2{tag}", tag=f"v2{tag}")
                nc.vector.tensor_copy(v2[:, 0:1], val_sb[:])
                nc.vector.tensor_copy(v2[:, 1:2], val_sb[:])
                b_ps = pss.tile([128, 2], f32, name=f"bps{tag}", tag="sm2", bufs=1)
                nc.tensor.matmul(b_ps[:], onesr_t[:], v2[:],
                                 start=True, stop=True)
                b_sb = pool.tile([128, 1], f32, name=f"bsb{tag}", tag=f"bsb{tag}")
                nc.vector.tensor_copy(b_sb[:], b_ps[:, 0:1])
                return b_sb

            # ---------- pooling phase: BL samples, stage-major in groups ----------
            nsamp = 1 if stage <= 2 else BL
            GD = min(4, nsamp)     # software-pipeline depth

            # preload every sample's feat tiles first (small DMAs ahead of
            # everything else in the queues)
            Bts = []
            for b in range(nsamp):
                B0 = pool.tile([128, C], f32r, name=f"B0_{b}", tag="B0",
                               bufs=nsamp)
                B1 = pool.tile([M - 128, C], f32r, name=f"B1_{b}", tag="B1",
                               bufs=nsamp)
                nc.sync.dma_start(B0[:], featT[b, 0:128, :])
                nc.sync.dma_start(B1[:], featT[b, 128:M, :])
                Bts.append((B0, B1))

            for g0 in range(0, nsamp, GD):
                gb = list(range(g0, min(g0 + GD, nsamp)))
                st = {b: {} for b in gb}

                # -- column sums
                for b in gb:
                    B0, B1 = Bts[b]
                    srow_ps = pss.tile([1, 2 * C], f32, name=f"srow{b}",
                                       tag="sm1")
                    nc.tensor.matmul(srow_ps[:, 0:C], ones_t[0:128, :], B0[:],
                                     start=True, stop=False)
                    nc.tensor.matmul(srow_ps[:, 0:C], ones_t[0:M - 128, :],
                                     B1[:], start=False, stop=True)
                    st[b]["srow"] = srow_ps
                for b in gb:
                    s_sb = pool.tile([1, C], f32r, name=f"s_sb{b}", tag="s_sb",
                                     bufs=GD)
                    t_sb = pool.tile([1, C], f32r, name=f"t_sb{b}", tag="t_sb",
                                     bufs=GD)
                    nc.scalar.copy(s_sb[:], st[b]["srow"][:, 0:C])
                    nc.scalar.mul(t_sb[:], st[b]["srow"][:, 0:C], -1.0 / M)
                    st[b]["s"], st[b]["t"] = s_sb, t_sb

                # -- G = A^T A - M xbar xbar^T
                for b in gb:
                    B0, B1 = Bts[b]
                    GS = psg.tile([128, 2 * C], f32, name=f"GS{b}", tag="Yp")
                    for r in range(2):
                        nc.tensor.matmul(GS[:, C * r:C * (r + 1)],
                                         B0[:, 128 * r:128 * (r + 1)], B0[:],
                                         start=True, stop=False)
                        nc.tensor.matmul(GS[:, C * r:C * (r + 1)],
                                         B1[:, 128 * r:128 * (r + 1)], B1[:],
                                         start=False, stop=False)
                        nc.tensor.matmul(GS[:, C * r:C * (r + 1)],
                                         st[b]["t"][:, 128 * r:128 * (r + 1)],
                                         st[b]["s"][:], start=False, stop=True)
                    st[b]["G"] = GS

                # -- trace -> 1/trG broadcast
                for b in gb:
                    scrS = pool.tile([128, 2 * C], f32r, name=f"scrS{b}",
                                     tag="scr", bufs=GD)
                    nc.vector.tensor_tensor(scrS[:], st[b]["G"][:],
                                            identS_t[:], AluOp.mult)
                    st[b]["scr"] = scrS
                for b in gb:
                    trrow_ps = pss.tile([1, 2 * C], f32, name=f"trrow{b}",
                                        tag="sm1")
                    nc.tensor.matmul(trrow_ps[:], ones_t[0:128, :],
                                     st[b]["scr"][:], start=True, stop=True)
                    st[b]["trrow"] = trrow_ps
                for b in gb:
                    tr_sb = pool.tile([1, 1], f32, name=f"tr_sb{b}",
                                      tag="tr_sb", bufs=GD)
                    nc.vector.tensor_reduce(out=tr_sb[:], in_=st[b]["trrow"][:],
                                            axis=mybir.AxisListType.X,
                                            op=AluOp.add)
                    inv_sb = pool.tile([1, 1], f32r, name=f"inv_sb{b}",
                                       tag="inv", bufs=GD)
                    nc.vector.reciprocal(inv_sb[:], tr_sb[:])
                    st[b]["invb"] = scalar_bcast(inv_sb, f"i{b}")

                # -- Y0 = G/trG ; T1 = 3I - Y0
                for b in gb:
                    Y0S = mats.tile([128, 2 * C], f32r, name=f"Y0S{b}",
                                    tag="Y0", bufs=GD)
                    nc.vector.tensor_scalar_mul(Y0S[:], st[b]["G"][:],
                                                st[b]["invb"][:])
                    st[b]["Y0"] = Y0S

                if stage == 1:
                    for r in range(2):
                        nc.sync.dma_start(
                            out[r, :, :],
                            st[gb[0]]["Y0"][:, C * r:C * (r + 1)].bitcast(f32))
                    break

                for b in gb:
                    T1S = mats.tile([128, 2 * C], f32r, name=f"T1S{b}",
                                    tag="T", bufs=GD)
                    nc.vector.scalar_tensor_tensor(
                        out=T1S[:], in0=st[b]["Y0"][:], scalar=-1.0,
                        in1=threeIS_t[:], op0=AluOp.mult, op1=AluOp.add)
                    st[b]["T1"] = T1S

                # -- iter1 products; Y1 = .5 Y0 T1 ; Z1 = .5 T1
                for b in gb:
                    YpS = psg.tile([128, 2 * C], f32, name=f"YpS{b}", tag="Yp")
                    mmp(YpS, st[b]["Y0"], st[b]["T1"])
                    st[b]["Yp"] = YpS
                for b in gb:
                    Y1S = mats.tile([128, 2 * C], f32r, name=f"Y1S{b}",
                                    tag="Y1", bufs=GD)
                    Z1S = mats.tile([128, 2 * C], f32r, name=f"Z1S{b}",
                                    tag="Z", bufs=GD)
                    nc.scalar.mul(Y1S[:], st[b]["Yp"][:], 0.5)
                    nc.scalar.mul(Z1S[:], st[b]["T1"][:], 0.5)
                    st[b]["Y1"], st[b]["Z1"] = Y1S, Z1S

                # -- iter2
                for b in gb:
                    PpS = psg.tile([128, 2 * C], f32, name=f"PpS{b}", tag="Yp")
                    mmp(PpS, st[b]["Z1"], st[b]["Y1"])
                    st[b]["Pp"] = PpS
                for b in gb:
                    T2S = mats.tile([128, 2 * C], f32r, name=f"T2S{b}",
                                    tag="T", bufs=GD)
                    nc.vector.scalar_tensor_tensor(
                        out=T2S[:], in0=st[b]["Pp"][:], scalar=-1.0,
                        in1=threeIS_t[:], op0=AluOp.mult, op1=AluOp.add)
                    st[b]["T2"] = T2S
                for b in gb:
                    Yp2S = psg.tile([128, 2 * C], f32, name=f"Yp2S{b}",
                                    tag="Yp")
                    mmp(Yp2S, st[b]["Y1"], st[b]["T2"])
                    st[b]["Yp2"] = Yp2S
                for b in gb:
                    Y2S = mats.tile([128, 2 * C], f32r, name=f"Y2S{b}",
                                    tag="Y2", bufs=GD)
                    nc.scalar.mul(Y2S[:], st[b]["Yp2"][:], 0.5)
                    st[b]["Y2"] = Y2S
                for b in gb:
                    ZpS = psg.tile([128, 2 * C], f32, name=f"ZpS{b}", tag="Yp")
                    mmp(ZpS, st[b]["T2"], st[b]["Z1"])
                    st[b]["Zp"] = ZpS
                for b in gb:
                    Z2S = mats.tile([128, 2 * C], f32r, name=f"Z2S{b}",
                                    tag="Z", bufs=GD)
                    nc.scalar.mul(Z2S[:], st[b]["Zp"][:], 0.5)
                    st[b]["Z2"] = Z2S

                # -- iter3 (Z dead)
                for b in gb:
                    Pp3S = psg.tile([128, 2 * C], f32, name=f"Pp3S{b}",
                                    tag="Yp")
                    mmp(Pp3S, st[b]["Z2"], st[b]["Y2"])
                    st[b]["Pp3"] = Pp3S
                for b in gb:
                    T3S = mats.tile([128, 2 * C], f32r, name=f"T3S{b}",
                                    tag="T", bufs=GD)
                    nc.vector.scalar_tensor_tensor(
                        out=T3S[:], in0=st[b]["Pp3"][:], scalar=-1.0,
                        in1=threeIS_t[:], op0=AluOp.mult, op1=AluOp.add)
                    st[b]["T3"] = T3S
                for b in gb:
                    Y3pS = psg.tile([128, 2 * C], f32, name=f"Y3pS{b}",
                                    tag="Yp")
                    mmp(Y3pS, st[b]["Y2"], st[b]["T3"])
                    st[b]["Y3p"] = Y3pS

                # -- flat-normalize + staging
                for b in gb:
                    sqS = pool.tile([128, 2 * C], f32r, name=f"sqS{b}",
                                    tag="scr", bufs=GD)
                    nc.scalar.activation(sqS[:], st[b]["Y3p"][:],
                                         mybir.ActivationFunctionType.Square)
                    st[b]["sq"] = sqS
                for b in gb:
                    ssqrow_ps = pss.tile([1, 2 * C], f32, name=f"ssqrow{b}",
                                         tag="sm1")
                    nc.tensor.matmul(ssqrow_ps[:], ones_t[0:128, :],
                                     st[b]["sq"][:], start=True, stop=True)
                    st[b]["ssqrow"] = ssqrow_ps
                for b in gb:
                    ssq_sb = pool.tile([1, 1], f32, name=f"ssq_sb{b}",
                                       tag="tr_sb", bufs=GD)
                    nc.vector.tensor_reduce(out=ssq_sb[:],
                                            in_=st[b]["ssqrow"][:],
                                            axis=mybir.AxisListType.X,
                                            op=AluOp.add)
                    sqr_sb = pool.tile([1, 1], f32, name=f"sqr_sb{b}",
                                       tag="sqr", bufs=GD)
                    nc.scalar.sqrt(sqr_sb[:], ssq_sb[:])
                    rsq_sb = pool.tile([1, 1], f32r, name=f"rsq_sb{b}",
                                       tag="inv", bufs=GD)
                    nc.vector.reciprocal(rsq_sb[:], sqr_sb[:])
                    st[b]["rsqb"] = scalar_bcast(rsq_sb, f"r{b}")
                for b in gb:
                    FS = mats.tile([128, 2 * C], f32r, name=f"FS{b}", tag="F",
                                   bufs=GD)
                    nc.vector.tensor_scalar_mul(FS[:], st[b]["Y3p"][:],
                                                st[b]["rsqb"][:])
                    st[b]["F"] = FS

                if stage == 2:
                    for r in range(2):
                        nc.sync.dma_start(
                            out[r, :, :],
                            st[gb[0]]["F"][:, C * r:C * (r + 1)].bitcast(f32))
                    break

                for b in gb:
                    for hh in range(2):
                        nc.sync.dma_start(
                            a2a_in_v[hh, b],                      # [p, j, i]
                            st[b]["F"][:, C * hh:C * (hh + 1)]
                                .rearrange("p (j i) -> p j i", j=8, i=32))

            # ---------- AllToAll ----------
            if stage >= 3:
                nc.gpsimd.collective_compute(
                    "AllToAll", AluOp.bypass, replica_groups=rg,
                    ins=[a2a_in.opt()], outs=[a2a_out.opt()])

            if stage == 3:
                tmp = bigpool.tile([128, 4096], f32, name="tmp")
                nc.sync.dma_start(tmp[:], a2a_out[:].bitcast(f32))
                nc.sync.dma_start(out[:], tmp[:])

            if stage >= 4:
                # ------- consumer: BIG [128, 4096], free = [h, s, b, i] -------
                BIG = bigpool.tile([128, 2 * 8 * BL * 32], f32r, name="BIG")
                a2a_out_v = a2a_out.flatten().rearrange(
                    "(s h p b i) -> h s p b i", s=8, h=2, p=128, b=BL, i=32)
                BIG_v = BIG[:].rearrange("p (h s b i) -> h s p b i",
                                         h=2, s=8, b=BL, i=32)
                for hh in range(2):
                    for s in range(8):
                        nc.sync.dma_start(BIG_v[hh, s], a2a_out_v[hh, s])

                # ------- projection: EMB[64, 512], W streamed -------
                EMB = pse.tile([64, E], f32, name="EMB")
                BIG_k = BIG[:].rearrange("p (h sb i) -> h i p sb",
                                         h=2, sb=64, i=32)
                wT_v = wT.rearrange("(c p) e -> c p e", p=128)  # [64,128,512]
                for c in range(NCH):
                    i_local, hh = c // 2, c % 2
                    wq = wpool.tile([128, E], f32r, name=f"wq{c}", tag="wq",
                                    bufs=32)
                    nc.sync.dma_start(wq[:], wT_v[c])
                    nc.tensor.matmul(
                        EMB[:], BIG_k[hh, i_local], wq[:],
                        start=(c == 0), stop=(c == NCH - 1))

                emb_sb = pool.tile([64, E], f32, name="emb_sb", tag="emb", bufs=1)
                nc.vector.tensor_copy(emb_sb[:], EMB[:])
                if stage == 4:
                    nc.sync.dma_start(out[:], emb_sb[:])

            if stage >= 5:
                nc.sync.dma_start(rs_in[:], emb_sb[:])

                # ------- ReduceScatter: [64, E] -> [8, E] -------
                nc.gpsimd.collective_compute(
                    "ReduceScatter", AluOp.add, replica_groups=rg,
                    ins=[rs_in.opt()], outs=[rs_out.opt()])

                # ------- BN fold + final L2 normalize -------
                e_sb = pool.tile([BL, E], f32, name="e_sb", tag="fin", bufs=1)
                nc.sync.dma_start(e_sb[:], rs_out[:])
                e_bn = pool.tile([BL, E], f32, name="e_bn", tag="fin2", bufs=1)
                nc.vector.tensor_tensor(e_bn[:], e_sb[:], bnsc_t[:], AluOp.mult)
                nc.vector.tensor_tensor(e_bn[:], e_bn[:], bnsh_t[:], AluOp.add)
                scr3 = pool.tile([BL, E], f32, name="scr3", tag="fin", bufs=1)
                nrm_sb = pool.tile([BL, 1], f32, name="nrm_sb", tag="nrm")
                nc.scalar.activation(
                    scr3[:], e_bn[:], mybir.ActivationFunctionType.Square,
                    accum_out=nrm_sb[:])
                nrms_sb = pool.tile([BL, 1], f32, name="nrms_sb", tag="nrms")
                nc.scalar.sqrt(nrms_sb[:], nrm_sb[:])
                rs_sb = pool.tile([BL, 1], f32, name="rs_sb", tag="nrmr")
                nc.vector.reciprocal(rs_sb[:], nrms_sb[:])
                e_fin = pool.tile([BL, E], f32, name="e_fin", tag="fin3", bufs=1)
                nc.vector.tensor_scalar_mul(e_fin[:], e_bn[:], rs_sb[:])
                nc.sync.dma_start(out[:], e_fin[:])

    _split_excess_waits(nc)
    return nc


def host_inputs(feat, W_proj, b_proj, bn_gamma, bn_beta, bn_mean, bn_var):
    """Build the 8 per-core input maps."""
    feat = np.ascontiguousarray(np.asarray(feat, dtype=np.float32))
    W_proj = np.asarray(W_proj, dtype=np.float32)
    featT = feat.reshape(B, C, M).transpose(0, 2, 1)          # [64, 196, 256]
    bnscale = (np.asarray(bn_gamma) /
               np.sqrt(np.asarray(bn_var) + BN_EPS)).astype(np.float32)
    bnshift = ((np.asarray(b_proj) - np.asarray(bn_mean)) * bnscale
               + np.asarray(bn_beta)).astype(np.float32)
    bnsc_rep = np.ascontiguousarray(np.broadcast_to(bnscale, (BL, E)))
    bnsh_rep = np.ascontiguousarray(np.broadcast_to(bnshift, (BL, E)))

    onesc = np.ones((128, 1), np.float32)
    onesr = np.ones((1, 128), np.float32)
    identS = np.zeros((128, 2 * C), np.float32)
    identS[:, 0:128] = np.eye(128, dtype=np.float32)          # chunk 0, j=p
    identS[:, C + 128:C + 256] = np.eye(128, dtype=np.float32)  # chunk 1, j=128+p

    in_maps = []
    for i in range(N_CORES):
        in_maps.append({
            "featT": np.ascontiguousarray(featT[i * BL:(i + 1) * BL]),
            "wT": np.ascontiguousarray(W_proj[:, KL * i:KL * (i + 1)].T),
            "onesc": onesc, "onesr": onesr, "identS": identS,
            "bnsc": bnsc_rep, "bnsh": bnsh_rep,
        })
    return in_maps


def kernel(feat, W_proj, b_proj, bn_gamma, bn_beta, bn_mean, bn_var):
    if "nc" not in _cache:
        _cache["nc"] = _build()
    nc = _cache["nc"]
    in_maps = host_inputs(feat, W_proj, b_proj, bn_gamma, bn_beta,
                          bn_mean, bn_var)
    last_err = None
    for _attempt in range(4):
        try:
            res = run_bass_kernel_spmd(nc, in_maps,
                                       core_ids=list(range(N_CORES)))
            break
        except Exception as e:  # transient NRT_EXEC_UNIT_UNRECOVERABLE flakes
            last_err = e
            import time as _time
            _time.sleep(2.0)
    else:
        raise last_err
    return np.concatenate([res.results[i]["out"] for i in range(N_CORES)],
                          axis=0)


# revision 17
# speedup vs baseline: 1.6047x; 1.0264x over previous
"""MPN-COV pooling + projection kernel for 8 Trainium2 NeuronCores.

Problem: nn_PillTeacher_48661979464182
  feat [64, 256, 14, 14] -> per-sample covariance + 3 Newton-Schulz sqrt
  iterations -> L2-normalize -> project with W_proj [512, 65536] -> BN -> L2.

Sharding:
  - Pooling phase: pure data parallel, 8 samples per core.
  - Projection: k-shard of W_proj (each core holds an 8192-wide slice of the
    contraction dim). AllToAll exchanges the normalized pooled matrices so
    every core gets its k-slice of all 64 samples; partial embeddings are
    summed with ReduceScatter back to the owning core of each sample.

Key tricks:
  - Every matrix in the Newton-Schulz iteration is a polynomial of the
    (symmetric) covariance -> symmetric -> matmul lhsT operands read the
    row-major tiles directly (no transposes on device; feat pre-transposed
    on host).
  - The final L2 normalization is invariant to any positive per-sample
    scale, so 1/M, 1/trace, sqrt(trY) and the 0.5 of the last NS Y-update
    all drop out.
  - fp32r (4x-rate fp32 matmul mode) for all matmuls.
  - BN + bias folded into a host-computed scale/shift.

Workarounds for this walrus build:
  - <=1 semaphore wait per instruction (_split_excess_waits post-pass).
  - no matmul with rhs free size 1 (scalar reductions go through [1, 256]
    row-sums + a free-axis reduce; scalar broadcasts use [1, 2] operands).
  - no tensor_tensor_reduce (mask-mult + tensor_reduce / activation instead).
"""
import sys
import numpy as np

sys.path.insert(0, "/opt/trn_rl_repo")

import concourse.bass as bass
import concourse.mybir as mybir
import concourse.tile as tile
import bass_rust
from concourse.bass_utils import run_bass_kernel_spmd

dt = mybir.dt

N_CORES = 8
B, C, H, W_SP = 64, 256, 14, 14
M = H * W_SP           # 196
E = 512
K = C * C              # 65536
BL = B // N_CORES      # 8 samples per core
KL = K // N_CORES      # 8192 contraction slice per core
BN_EPS = 1e-5

_cache = {}


def _split_excess_waits(nc, max_waits=1):
    """walrus in this env rejects >1 semaphore wait per instruction; hoist
    excess waits onto preceding NoOps on the same engine."""
    for fn in nc.m.functions:
        for bb in fn.blocks:
            new_insts = []
            for inst in bb.instructions:
                si = inst.sync_info
                if si is not None and si.on_wait and len(si.on_wait) > max_waits:
                    waits = list(si.on_wait)
                    chunks = [waits[i:i + max_waits]
                              for i in range(0, len(waits), max_waits)]
                    for chunk in chunks[:-1]:
                        nop = mybir.InstNoOp(
                            name=nc.get_next_instruction_name(), ins=[], outs=[],
                            engine=inst.engine)
                        nop.sync_info = bass_rust.SyncInfo(on_wait=chunk,
                                                           on_update=[])
                        new_insts.append(nop)
                    si.on_wait = chunks[-1]
                new_insts.append(inst)
            bb.instructions = new_insts


def _build(stage=5):
    """stage: 1=Y0 dump, 2=F dump, 3=a2a_out dump, 4=emb partial dump,
    5=full kernel.

    All 256x256 matrices use a stacked-pair layout: S[p, r*256 + j] =
    X[128*r + p, j] -- one [128, 512] tile per matrix, so every elementwise
    op is a single instruction and every PSUM product fills one full bank."""
    f32, f32r = dt.float32, dt.float32r
    nc = bass.Bass("TRN2", target_bir_lowering=False, debug=False,
                   num_devices=N_CORES)

    featT = nc.dram_tensor("featT", [BL, M, C], f32r, kind="ExternalInput")
    onesc = nc.dram_tensor("onesc", [128, 1], f32r, kind="ExternalInput")
    onesr = nc.dram_tensor("onesr", [1, 128], f32r, kind="ExternalInput")
    ident3 = nc.dram_tensor("threeIS", [128, 2 * C], f32, kind="ExternalInput")
    if stage >= 4:
        wT = nc.dram_tensor("wT", [KL, E], f32r, kind="ExternalInput")
    if stage >= 5:
        bnsc = nc.dram_tensor("bnsc", [BL, E], f32, kind="ExternalInput")
        bnsh = nc.dram_tensor("bnsh", [BL, E], f32, kind="ExternalInput")
        out = nc.dram_tensor("out", [BL, E], f32, kind="ExternalOutput")
    elif stage <= 2:
        out = nc.dram_tensor("dbg", [2, 128, C], f32, kind="ExternalOutput")
    elif stage == 3:
        out = nc.dram_tensor("dbg", [128, 4096], f32, kind="ExternalOutput")
    else:
        out = nc.dram_tensor("dbg", [64, E], f32, kind="ExternalOutput")

    rg = [list(range(N_CORES))]
    AluOp = mybir.AluOpType
    NCH = KL // 128        # 64 k-chunks for the projection

    lp = nc.allow_low_precision(reason="f32r intermediates carry fp32 bits")
    lp.__enter__()
    with tile.TileContext(nc) as tc:
        with (
            tc.tile_pool(name="consts", bufs=1) as cpool,
            tc.tile_pool(name="wbuf", bufs=1) as wpool,
            tc.tile_pool(name="big", bufs=1) as bigpool,
            tc.tile_pool(name="work", bufs=3) as pool,
            tc.tile_pool(name="mats", bufs=2) as mats,
            tc.tile_pool(name="pss", bufs=2, space="PSUM") as pss,
            tc.tile_pool(name="psg", bufs=5, space="PSUM") as psg,
            tc.tile_pool(name="dram", bufs=1, space="DRAM") as dram,
        ):
            # ---------- constants ----------
            ones_t = cpool.tile([128, 1], f32r, name="ones_t")
            nc.sync.dma_start(ones_t[:], onesc[:])
            onesr_t = cpool.tile([1, 128], f32r, name="onesr_t")
            nc.sync.dma_start(onesr_t[:], onesr[:])
            threeIS_t = cpool.tile([128, 2 * C], f32, name="threeIS_t")
            nc.sync.dma_start(threeIS_t[:], ident3[:])
            if stage >= 5:
                bnsc_t = cpool.tile([BL, E], f32, name="bnsc_t")
                bnsh_t = cpool.tile([BL, E], f32, name="bnsh_t")
                nc.sync.dma_start(bnsc_t[:], bnsc[:])
                nc.sync.dma_start(bnsh_t[:], bnsh[:])

            # ---------- DRAM staging for collectives ----------
            if stage >= 3:
                # a2a flat layout: [j(8), h(2), p(128), b_l(8), i0(32)]
                a2a_in = dram.tile([128, 4096], f32r, name="a2a_in")
                a2a_out = dram.tile([128, 4096], f32r, name="a2a_out")
                a2a_in_v = a2a_in.flatten().rearrange(
                    "(j h p b i) -> h b p j i", j=8, h=2, p=128, b=BL, i=32)
            if stage >= 5:
                rs_in = dram.tile([B, E], f32, name="rs_in")
                rs_out = dram.tile([BL, E], f32, name="rs_out")

            def mmp(outS, AS, BS):
                """outS = A @ B for symmetric A, all in stacked-pair layout."""
                for r in range(2):
                    for kc in range(2):
                        nc.tensor.matmul(
                            outS[:, C * r:C * (r + 1)],
                            AS[:, C * kc + 128 * r:C * kc + 128 * r + 128],
                            BS[:, C * kc:C * (kc + 1)],
                            start=(kc == 0), stop=(kc == 1))

            def scalar_bcast(val_sb, tag):
                """[1,1] f32r scalar -> [128,1] f32 SBUF (via N=2 matmul)."""
                v2 = pool.tile([1, 2], f32r, name=f"v2{tag}", tag=f"v2{tag}")
                nc.vector.tensor_copy(v2[:, 0:1], val_sb[:])
                nc.vector.tensor_copy(v2[:, 1:2], val_sb[:])
                b_ps = pss.tile([128, 2], f32, name=f"bps{tag}", tag="sm2", bufs=1)
                nc.tensor.matmul(b_ps[:], onesr_t[:], v2[:],
                                 start=True, stop=True)
                b_sb = pool.tile([128, 1], f32, name=f"bsb{tag}", tag=f"bsb{tag}")
                nc.vector.tensor_copy(b_sb[:], b_ps[:, 0:1])
                return b_sb

            # ---------- pooling phase: BL samples, stage-major in groups ----------
            nsamp = 1 if stage <= 2 else BL
            GD = min(4, nsamp)     # software-pipeline depth

            # preload every sample's feat tiles first (small DMAs ahead of
            # everything else in the queues)
            Bts = []
            for b in range(nsamp):
                B0 = pool.tile([128, C], f32r, name=f"B0_{b}", tag="B0",
                               bufs=nsamp)
                B1 = pool.tile([M - 128, C], f32r, name=f"B1_{b}", tag="B1",
                               bufs=nsamp)
                nc.sync.dma_start(B0[:], featT[b, 0:128, :])
                nc.sync.dma_start(B1[:], featT[b, 128:M, :])
                Bts.append((B0, B1))

            for g0 in range(0, nsamp, GD):
                gb = list(range(g0, min(g0 + GD, nsamp)))
                st = {b: {} for b in gb}

                # -- column sums
                for b in gb:
                    B0, B1 = Bts[b]
                    srow_ps = pss.tile([1, 2 * C], f32, name=f"srow{b}",
                                       tag="sm1")
                    nc.tensor.matmul(srow_ps[:, 0:C], ones_t[0:128, :], B0[:],
                                     start=True, stop=False)
                    nc.tensor.matmul(srow_ps[:, 0:C], ones_t[0:M - 128, :],
                                     B1[:], start=False, stop=True)
                    st[b]["srow"] = srow_ps
                for b in gb:
                    s_sb = pool.tile([1, C], f32r, name=f"s_sb{b}", tag="s_sb",
                                     bufs=GD)
                    t_sb = pool.tile([1, C], f32r, name=f"t_sb{b}", tag="t_sb",
                                     bufs=GD)
                    nc.scalar.copy(s_sb[:], st[b]["srow"][:, 0:C])
                    nc.scalar.mul(t_sb[:], st[b]["srow"][:, 0:C], -1.0 / M)
                    st[b]["s"], st[b]["t"] = s_sb, t_sb
                    # squares for the trace (parallel to G):
                    # tr(G) = ||A||_F^2 - ||srow||^2 / M
                    sqB0 = pool.tile([128, C], f32r, name=f"sqB0_{b}",
                                     tag="sqB0", bufs=GD)
                    sqB1 = pool.tile([M - 128, C], f32r, name=f"sqB1_{b}",
                                     tag="sqB1", bufs=GD)
                    s2 = pool.tile([1, C], f32, name=f"s2_{b}", tag="s2",
                                   bufs=GD)
                    B0, B1 = Bts[b]
                    nc.scalar.activation(sqB0[:], B0[:],
                                         mybir.ActivationFunctionType.Square)
                    nc.scalar.activation(sqB1[:], B1[:],
                                         mybir.ActivationFunctionType.Square)
                    nc.scalar.activation(s2[:], st[b]["srow"][:, 0:C],
                                         mybir.ActivationFunctionType.Square)
                    st[b]["sqB0"], st[b]["sqB1"], st[b]["s2"] = sqB0, sqB1, s2
                for b in gb:
                    trrow_ps = pss.tile([1, 2 * C], f32, name=f"trrow{b}",
                                        tag="sm1")
                    nc.tensor.matmul(trrow_ps[:, 0:C], ones_t[0:128, :],
                                     st[b]["sqB0"][:], start=True, stop=False)
                    nc.tensor.matmul(trrow_ps[:, 0:C], ones_t[0:M - 128, :],
                                     st[b]["sqB1"][:], start=False, stop=True)
                    st[b]["trrow"] = trrow_ps
                for b in gb:
                    asum = pool.tile([1, 1], f32, name=f"asum{b}", tag="asum",
                                     bufs=GD)
                    nc.vector.tensor_reduce(out=asum[:],
                                            in_=st[b]["trrow"][:, 0:C],
                                            axis=mybir.AxisListType.X,
                                            op=AluOp.add)
                    s2sum = pool.tile([1, 1], f32, name=f"s2sum{b}", tag="s2s",
                                      bufs=GD)
                    nc.vector.tensor_reduce(out=s2sum[:], in_=st[b]["s2"][:],
                                            axis=mybir.AxisListType.X,
                                            op=AluOp.add)
                    tr_sb = pool.tile([1, 1], f32, name=f"tr_sb{b}",
                                      tag="tr_sb", bufs=GD)
                    nc.vector.scalar_tensor_tensor(
                        out=tr_sb[:], in0=s2sum[:], scalar=-1.0 / M,
                        in1=asum[:], op0=AluOp.mult, op1=AluOp.add)
                    inv_sb = pool.tile([1, 1], f32r, name=f"inv_sb{b}",
                                       tag="inv", bufs=GD)
                    nc.vector.reciprocal(inv_sb[:], tr_sb[:])
                    st[b]["invb"] = scalar_bcast(inv_sb, f"i{b}")

                # -- G = A^T A - M xbar xbar^T
                for b in gb:
                    B0, B1 = Bts[b]
                    GS = psg.tile([128, 2 * C], f32, name=f"GS{b}", tag="Yp")
                    for r in range(2):
                        nc.tensor.matmul(GS[:, C * r:C * (r + 1)],
                                         B0[:, 128 * r:128 * (r + 1)], B0[:],
                                         start=True, stop=False)
                        nc.tensor.matmul(GS[:, C * r:C * (r + 1)],
                                         B1[:, 128 * r:128 * (r + 1)], B1[:],
                                         start=False, stop=False)
                        nc.tensor.matmul(GS[:, C * r:C * (r + 1)],
                                         st[b]["t"][:, 128 * r:128 * (r + 1)],
                                         st[b]["s"][:], start=False, stop=True)
                    st[b]["G"] = GS

                # -- Y0 = G/trG ; T1 = 3I - Y0
                for b in gb:
                    Y0S = mats.tile([128, 2 * C], f32r, name=f"Y0S{b}",
                                    tag="Y0", bufs=GD)
                    nc.vector.tensor_scalar_mul(Y0S[:], st[b]["G"][:],
                                                st[b]["invb"][:])
                    st[b]["Y0"] = Y0S

                if stage == 1:
                    for r in range(2):
                        nc.sync.dma_start(
                            out[r, :, :],
                            st[gb[0]]["Y0"][:, C * r:C * (r + 1)].bitcast(f32))
                    break

                for b in gb:
                    T1S = mats.tile([128, 2 * C], f32r, name=f"T1S{b}",
                                    tag="T", bufs=GD)
                    nc.vector.scalar_tensor_tensor(
                        out=T1S[:], in0=st[b]["Y0"][:], scalar=-1.0,
                        in1=threeIS_t[:], op0=AluOp.mult, op1=AluOp.add)
                    st[b]["T1"] = T1S

                # -- iter1 products; Y1 = .5 Y0 T1 ; Z1 = .5 T1
                for b in gb:
                    YpS = psg.tile([128, 2 * C], f32, name=f"YpS{b}", tag="Yp")
                    mmp(YpS, st[b]["Y0"], st[b]["T1"])
                    st[b]["Yp"] = YpS
                for b in gb:
                    Y1S = mats.tile([128, 2 * C], f32r, name=f"Y1S{b}",
                                    tag="Y1", bufs=GD)
                    Z1S = mats.tile([128, 2 * C], f32r, name=f"Z1S{b}",
                                    tag="Z", bufs=GD)
                    nc.scalar.mul(Y1S[:], st[b]["Yp"][:], 0.5)
                    nc.scalar.mul(Z1S[:], st[b]["T1"][:], 0.5)
                    st[b]["Y1"], st[b]["Z1"] = Y1S, Z1S

                # -- iter2
                for b in gb:
                    PpS = psg.tile([128, 2 * C], f32, name=f"PpS{b}", tag="Yp")
                    mmp(PpS, st[b]["Z1"], st[b]["Y1"])
                    st[b]["Pp"] = PpS
                for b in gb:
                    T2S = mats.tile([128, 2 * C], f32r, name=f"T2S{b}",
                                    tag="T", bufs=GD)
                    nc.vector.scalar_tensor_tensor(
                        out=T2S[:], in0=st[b]["Pp"][:], scalar=-1.0,
                        in1=threeIS_t[:], op0=AluOp.mult, op1=AluOp.add)
                    st[b]["T2"] = T2S
                for b in gb:
                    Yp2S = psg.tile([128, 2 * C], f32, name=f"Yp2S{b}",
                                    tag="Yp")
                    mmp(Yp2S, st[b]["Y1"], st[b]["T2"])
                    st[b]["Yp2"] = Yp2S
                for b in gb:
                    Y2S = mats.tile([128, 2 * C], f32r, name=f"Y2S{b}",
                                    tag="Y2", bufs=GD)
                    nc.scalar.mul(Y2S[:], st[b]["Yp2"][:], 0.5)
                    st[b]["Y2"] = Y2S
                for b in gb:
                    ZpS = psg.tile([128, 2 * C], f32, name=f"ZpS{b}", tag="Yp")
                    mmp(ZpS, st[b]["T2"], st[b]["Z1"])
                    st[b]["Zp"] = ZpS
                for b in gb:
                    Z2S = mats.tile([128, 2 * C], f32r, name=f"Z2S{b}",
                                    tag="Z", bufs=GD)
                    nc.scalar.mul(Z2S[:], st[b]["Zp"][:], 0.5)
                    st[b]["Z2"] = Z2S

                # -- iter3 (Z dead)
                for b in gb:
                    Pp3S = psg.tile([128, 2 * C], f32, name=f"Pp3S{b}",
                                    tag="Yp")
                    mmp(Pp3S, st[b]["Z2"], st[b]["Y2"])
                    st[b]["Pp3"] = Pp3S
                for b in gb:
                    T3S = mats.tile([128, 2 * C], f32r, name=f"T3S{b}",
                                    tag="T", bufs=GD)
                    nc.vector.scalar_tensor_tensor(
                        out=T3S[:], in0=st[b]["Pp3"][:], scalar=-1.0,
                        in1=threeIS_t[:], op0=AluOp.mult, op1=AluOp.add)
                    st[b]["T3"] = T3S
                for b in gb:
                    Y3pS = psg.tile([128, 2 * C], f32, name=f"Y3pS{b}",
                                    tag="Yp")
                    mmp(Y3pS, st[b]["Y2"], st[b]["T3"])
                    st[b]["Y3p"] = Y3pS

                # -- flat-normalize + staging
                for b in gb:
                    sqS = pool.tile([128, 2 * C], f32r, name=f"sqS{b}",
                                    tag="scr", bufs=GD)
                    nc.scalar.activation(sqS[:], st[b]["Y3p"][:],
                                         mybir.ActivationFunctionType.Square)
                    st[b]["sq"] = sqS
                for b in gb:
                    ssqrow_ps = pss.tile([1, 2 * C], f32, name=f"ssqrow{b}",
                                         tag="sm1")
                    nc.tensor.matmul(ssqrow_ps[:], ones_t[0:128, :],
                                     st[b]["sq"][:], start=True, stop=True)
                    st[b]["ssqrow"] = ssqrow_ps
                for b in gb:
                    ssq_sb = pool.tile([1, 1], f32, name=f"ssq_sb{b}",
                                       tag="tr_sb", bufs=GD)
                    nc.vector.tensor_reduce(out=ssq_sb[:],
                                            in_=st[b]["ssqrow"][:],
                                            axis=mybir.AxisListType.X,
                                            op=AluOp.add)
                    sqr_sb = pool.tile([1, 1], f32, name=f"sqr_sb{b}",
                                       tag="sqr", bufs=GD)
                    nc.scalar.sqrt(sqr_sb[:], ssq_sb[:])
                    rsq_sb = pool.tile([1, 1], f32r, name=f"rsq_sb{b}",
                                       tag="inv", bufs=GD)
                    nc.vector.reciprocal(rsq_sb[:], sqr_sb[:])
                    st[b]["rsqb"] = scalar_bcast(rsq_sb, f"r{b}")
                for b in gb:
                    FS = mats.tile([128, 2 * C], f32r, name=f"FS{b}", tag="F",
                                   bufs=GD)
                    nc.vector.tensor_scalar_mul(FS[:], st[b]["Y3p"][:],
                                                st[b]["rsqb"][:])
                    st[b]["F"] = FS

                if stage == 2:
                    for r in range(2):
                        nc.sync.dma_start(
                            out[r, :, :],
                            st[gb[0]]["F"][:, C * r:C * (r + 1)].bitcast(f32))
                    break

                for b in gb:
                    for hh in range(2):
                        nc.sync.dma_start(
                            a2a_in_v[hh, b],                      # [p, j, i]
                            st[b]["F"][:, C * hh:C * (hh + 1)]
                                .rearrange("p (j i) -> p j i", j=8, i=32))

            # ---------- AllToAll ----------
            if stage >= 3:
                nc.gpsimd.collective_compute(
                    "AllToAll", AluOp.bypass, replica_groups=rg,
                    ins=[a2a_in.opt()], outs=[a2a_out.opt()])

            if stage == 3:
                tmp = bigpool.tile([128, 4096], f32, name="tmp")
                nc.sync.dma_start(tmp[:], a2a_out[:].bitcast(f32))
                nc.sync.dma_start(out[:], tmp[:])

            if stage >= 4:
                # ------- consumer: BIG [128, 4096], free = [h, s, b, i] -------
                BIG = bigpool.tile([128, 2 * 8 * BL * 32], f32r, name="BIG")
                a2a_out_v = a2a_out.flatten().rearrange(
                    "(s h p b i) -> h s p b i", s=8, h=2, p=128, b=BL, i=32)
                BIG_v = BIG[:].rearrange("p (h s b i) -> h s p b i",
                                         h=2, s=8, b=BL, i=32)
                for hh in range(2):
                    for s in range(8):
                        nc.sync.dma_start(BIG_v[hh, s], a2a_out_v[hh, s])

                # ------- projection: EMB[64, 512], W streamed -------
                EMB = pss.tile([64, E], f32, name="EMB", tag="sm1")
                BIG_k = BIG[:].rearrange("p (h sb i) -> h i p sb",
                                         h=2, sb=64, i=32)
                wT_v = wT.rearrange("(c p) e -> c p e", p=128)  # [64,128,512]
                for c in range(NCH):
                    i_local, hh = c // 2, c % 2
                    wq = wpool.tile([128, E], f32r, name=f"wq{c}", tag="wq",
                                    bufs=40)
                    nc.sync.dma_start(wq[:], wT_v[c])
                    nc.tensor.matmul(
                        EMB[:], BIG_k[hh, i_local], wq[:],
                        start=(c == 0), stop=(c == NCH - 1))

                emb_sb = pool.tile([64, E], f32, name="emb_sb", tag="emb", bufs=1)
                nc.vector.tensor_copy(emb_sb[:], EMB[:])
                if stage == 4:
                    nc.sync.dma_start(out[:], emb_sb[:])

            if stage >= 5:
                nc.sync.dma_start(rs_in[:], emb_sb[:])

                # ------- ReduceScatter: [64, E] -> [8, E] -------
                nc.gpsimd.collective_compute(
                    "ReduceScatter", AluOp.add, replica_groups=rg,
                    ins=[rs_in.opt()], outs=[rs_out.opt()])

                # ------- BN fold + final L2 normalize -------
                e_sb = pool.tile([BL, E], f32, name="e_sb", tag="fin", bufs=1)
                nc.sync.dma_start(e_sb[:], rs_out[:])
                e_bn = pool.tile([BL, E], f32, name="e_bn", tag="fin2", bufs=1)
                nc.vector.tensor_tensor(e_bn[:], e_sb[:], bnsc_t[:], AluOp.mult)
                nc.vector.tensor_tensor(e_bn[:], e_bn[:], bnsh_t[:], AluOp.add)
                scr3 = pool.tile([BL, E], f32, name="scr3", tag="fin", bufs=1)
                nrm_sb = pool.tile([BL, 1], f32, name="nrm_sb", tag="nrm")
                nc.scalar.activation(
                    scr3[:], e_bn[:], mybir.ActivationFunctionType.Square,
                    accum_out=nrm_sb[:])
                nrms_sb = pool.tile([BL, 1], f32, name="nrms_sb", tag="nrms")
                nc.scalar.sqrt(nrms_sb[:], nrm_sb[:])
                rs_sb = pool.tile([BL, 1], f32, name="rs_sb", tag="nrmr")
                nc.vector.reciprocal(rs_sb[:], nrms_sb[:])
                e_fin = pool.tile([BL, E], f32, name="e_fin", tag="fin3", bufs=1)
                nc.vector.tensor_scalar_mul(e_fin[:], e_bn[:], rs_sb[:])
                nc.sync.dma_start(out[:], e_fin[:])

    _split_excess_waits(nc)
    return nc


def host_inputs(feat, W_proj, b_proj, bn_gamma, bn_beta, bn_mean, bn_var):
    """Build the 8 per-core input maps."""
    feat = np.ascontiguousarray(np.asarray(feat, dtype=np.float32))
    W_proj = np.asarray(W_proj, dtype=np.float32)
    featT = feat.reshape(B, C, M).transpose(0, 2, 1)          # [64, 196, 256]
    bnscale = (np.asarray(bn_gamma) /
               np.sqrt(np.asarray(bn_var) + BN_EPS)).astype(np.float32)
    bnshift = ((np.asarray(b_proj) - np.asarray(bn_mean)) * bnscale
               + np.asarray(bn_beta)).astype(np.float32)
    bnsc_rep = np.ascontiguousarray(np.broadcast_to(bnscale, (BL, E)))
    bnsh_rep = np.ascontiguousarray(np.broadcast_to(bnshift, (BL, E)))

    onesc = np.ones((128, 1), np.float32)
    onesr = np.ones((1, 128), np.float32)
    threeIS = np.zeros((128, 2 * C), np.float32)
    threeIS[:, 0:128] = 3.0 * np.eye(128, dtype=np.float32)
    threeIS[:, C + 128:C + 256] = 3.0 * np.eye(128, dtype=np.float32)

    in_maps = []
    for i in range(N_CORES):
        in_maps.append({
            "featT": np.ascontiguousarray(featT[i * BL:(i + 1) * BL]),
            "wT": np.ascontiguousarray(W_proj[:, KL * i:KL * (i + 1)].T),
            "onesc": onesc, "onesr": onesr, "threeIS": threeIS,
            "bnsc": bnsc_rep, "bnsh": bnsh_rep,
        })
    return in_maps


def kernel(feat, W_proj, b_proj, bn_gamma, bn_beta, bn_mean, bn_var):
    if "nc" not in _cache:
        _cache["nc"] = _build()
    nc = _cache["nc"]
    in_maps = host_inputs(feat, W_proj, b_proj, bn_gamma, bn_beta,
                          bn_mean, bn_var)
    last_err = None
    for _attempt in range(4):
        try:
            res = run_bass_kernel_spmd(nc, in_maps,
                                       core_ids=list(range(N_CORES)))
            break
        except Exception as e:  # transient NRT_EXEC_UNIT_UNRECOVERABLE flakes
            last_err = e
            import time as _time
            _time.sleep(2.0)
    else:
        raise last_err
    return np.concatenate([res.results[i]["out"] for i in range(N_CORES)],
                          axis=0)


# revision 20
# speedup vs baseline: 1.7690x; 1.1024x over previous
"""MPN-COV pooling + projection kernel for 8 Trainium2 NeuronCores.

Problem: nn_PillTeacher_48661979464182
  feat [64, 256, 14, 14] -> per-sample covariance + 3 Newton-Schulz sqrt
  iterations -> L2-normalize -> project with W_proj [512, 65536] -> BN -> L2.

Sharding:
  - Pooling phase: pure data parallel, 8 samples per core.
  - Projection: k-shard of W_proj (each core holds an 8192-wide slice of the
    contraction dim). AllToAll exchanges the normalized pooled matrices so
    every core gets its k-slice of all 64 samples; partial embeddings are
    summed with ReduceScatter back to the owning core of each sample.

Key tricks:
  - Every matrix in the Newton-Schulz iteration is a polynomial of the
    (symmetric) covariance -> symmetric -> matmul lhsT operands read the
    row-major tiles directly (no transposes on device; feat pre-transposed
    on host).
  - The final L2 normalization is invariant to any positive per-sample
    scale, so 1/M, 1/trace, sqrt(trY) and the 0.5 of the last NS Y-update
    all drop out.
  - fp32r (4x-rate fp32 matmul mode) for all matmuls.
  - BN + bias folded into a host-computed scale/shift.

Workarounds for this walrus build:
  - <=1 semaphore wait per instruction (_split_excess_waits post-pass).
  - no matmul with rhs free size 1 (scalar reductions go through [1, 256]
    row-sums + a free-axis reduce; scalar broadcasts use [1, 2] operands).
  - no tensor_tensor_reduce (mask-mult + tensor_reduce / activation instead).
"""
import sys
import numpy as np

sys.path.insert(0, "/opt/trn_rl_repo")

import concourse.bass as bass
import concourse.mybir as mybir
import concourse.tile as tile
import bass_rust
from concourse.bass_utils import run_bass_kernel_spmd

dt = mybir.dt

N_CORES = 8
B, C, H, W_SP = 64, 256, 14, 14
M = H * W_SP           # 196
E = 512
K = C * C              # 65536
BL = B // N_CORES      # 8 samples per core
KL = K // N_CORES      # 8192 contraction slice per core
BN_EPS = 1e-5

_cache = {}


def _split_excess_waits(nc, max_waits=1):
    """walrus in this env rejects >1 semaphore wait per instruction; hoist
    excess waits onto preceding NoOps on the same engine."""
    for fn in nc.m.functions:
        for bb in fn.blocks:
            new_insts = []
            for inst in bb.instructions:
                si = inst.sync_info
                if si is not None and si.on_wait and len(si.on_wait) > max_waits:
                    waits = list(si.on_wait)
                    chunks = [waits[i:i + max_waits]
                              for i in range(0, len(waits), max_waits)]
                    for chunk in chunks[:-1]:
                        nop = mybir.InstNoOp(
                            name=nc.get_next_instruction_name(), ins=[], outs=[],
                            engine=inst.engine)
                        nop.sync_info = bass_rust.SyncInfo(on_wait=chunk,
                                                           on_update=[])
                        new_insts.append(nop)
                    si.on_wait = chunks[-1]
                new_insts.append(inst)
            bb.instructions = new_insts


def _build(stage=5):
    """stage: 1=Y0 dump, 2=F dump, 3=a2a_out dump, 4=emb partial dump,
    5=full kernel.

    All 256x256 matrices use a stacked-pair layout: S[p, r*256 + j] =
    X[128*r + p, j] -- one [128, 512] tile per matrix, so every elementwise
    op is a single instruction and every PSUM product fills one full bank."""
    f32, f32r = dt.float32, dt.float32r
    nc = bass.Bass("TRN2", target_bir_lowering=False, debug=False,
                   num_devices=N_CORES)

    featT = nc.dram_tensor("featT", [BL, M, C], f32r, kind="ExternalInput")
    onesc = nc.dram_tensor("onesc", [128, 1], f32r, kind="ExternalInput")
    onesr = nc.dram_tensor("onesr", [1, 128], f32r, kind="ExternalInput")
    ident3 = nc.dram_tensor("threeIS", [128, 2 * C], f32, kind="ExternalInput")
    if stage >= 4:
        wT = nc.dram_tensor("wT", [KL, E], f32r, kind="ExternalInput")
    if stage >= 5:
        bnsc = nc.dram_tensor("bnsc", [BL, E], f32, kind="ExternalInput")
        bnsh = nc.dram_tensor("bnsh", [BL, E], f32, kind="ExternalInput")
        out = nc.dram_tensor("out", [BL, E], f32, kind="ExternalOutput")
    elif stage <= 2:
        out = nc.dram_tensor("dbg", [2, 128, C], f32, kind="ExternalOutput")
    elif stage == 3:
        out = nc.dram_tensor("dbg", [128, 4096], f32, kind="ExternalOutput")
    else:
        out = nc.dram_tensor("dbg", [64, E], f32, kind="ExternalOutput")

    rg = [list(range(N_CORES))]
    AluOp = mybir.AluOpType
    NCH = KL // 128        # 64 k-chunks for the projection

    lp = nc.allow_low_precision(reason="f32r intermediates carry fp32 bits")
    lp.__enter__()
    with tile.TileContext(nc) as tc:
        with (
            tc.tile_pool(name="consts", bufs=1) as cpool,
            tc.tile_pool(name="wbuf", bufs=1) as wpool,
            tc.tile_pool(name="big", bufs=1) as bigpool,
            tc.tile_pool(name="work", bufs=3) as pool,
            tc.tile_pool(name="mats", bufs=2) as mats,
            tc.tile_pool(name="pss", bufs=2, space="PSUM") as pss,
            tc.tile_pool(name="psg", bufs=5, space="PSUM") as psg,
            tc.tile_pool(name="dram", bufs=1, space="DRAM") as dram,
        ):
            # ---------- constants ----------
            ones_t = cpool.tile([128, 1], f32r, name="ones_t")
            nc.sync.dma_start(ones_t[:], onesc[:])
            onesr_t = cpool.tile([1, 128], f32r, name="onesr_t")
            nc.sync.dma_start(onesr_t[:], onesr[:])
            threeIS_t = cpool.tile([128, 2 * C], f32, name="threeIS_t")
            nc.sync.dma_start(threeIS_t[:], ident3[:])
            if stage >= 5:
                bnsc_t = cpool.tile([BL, E], f32, name="bnsc_t")
                bnsh_t = cpool.tile([BL, E], f32, name="bnsh_t")
                nc.sync.dma_start(bnsc_t[:], bnsc[:])
                nc.sync.dma_start(bnsh_t[:], bnsh[:])

            # ---------- DRAM staging for collectives ----------
            if stage >= 3:
                # two half-batch exchanges: half q carries samples 4q..4q+3
                # flat layout per half: [j(8), h(2), p(128), b_l(4), i0(32)]
                a2a_in = [dram.tile([128, 2048], f32r, name=f"a2a_in{q}")
                          for q in range(2)]
                a2a_out = [dram.tile([128, 2048], f32r, name=f"a2a_out{q}")
                           for q in range(2)]
                a2a_in_v = [a2a_in[q].flatten().rearrange(
                    "(j h p b i) -> h b p j i", j=8, h=2, p=128, b=BL // 2,
                    i=32) for q in range(2)]
            if stage >= 5:
                rs_in = dram.tile([B, E], f32, name="rs_in")
                rs_out = dram.tile([BL, E], f32, name="rs_out")

            def mmp(outS, AS, BS):
                """outS = A @ B for symmetric A, all in stacked-pair layout."""
                for r in range(2):
                    for kc in range(2):
                        nc.tensor.matmul(
                            outS[:, C * r:C * (r + 1)],
                            AS[:, C * kc + 128 * r:C * kc + 128 * r + 128],
                            BS[:, C * kc:C * (kc + 1)],
                            start=(kc == 0), stop=(kc == 1))

            def scalar_bcast(val_sb, tag):
                """[1,1] f32r scalar -> [128,1] f32 SBUF (via N=2 matmul)."""
                v2 = pool.tile([1, 2], f32r, name=f"v2{tag}", tag=f"v2{tag}")
                nc.vector.tensor_copy(v2[:, 0:1], val_sb[:])
                nc.vector.tensor_copy(v2[:, 1:2], val_sb[:])
                b_ps = pss.tile([128, 2], f32, name=f"bps{tag}", tag="sm2", bufs=1)
                nc.tensor.matmul(b_ps[:], onesr_t[:], v2[:],
                                 start=True, stop=True)
                b_sb = pool.tile([128, 1], f32, name=f"bsb{tag}", tag=f"bsb{tag}")
                nc.vector.tensor_copy(b_sb[:], b_ps[:, 0:1])
                return b_sb

            # ---------- pooling phase: BL samples, stage-major in groups ----------
            nsamp = 1 if stage <= 2 else BL
            GD = min(4, nsamp)     # software-pipeline depth

            # preload every sample's feat tiles first (small DMAs ahead of
            # everything else in the queues)
            Bts = []
            for b in range(nsamp):
                B0 = pool.tile([128, C], f32r, name=f"B0_{b}", tag="B0",
                               bufs=nsamp)
                B1 = pool.tile([M - 128, C], f32r, name=f"B1_{b}", tag="B1",
                               bufs=nsamp)
                nc.sync.dma_start(B0[:], featT[b, 0:128, :])
                nc.sync.dma_start(B1[:], featT[b, 128:M, :])
                Bts.append((B0, B1))

            for g0 in range(0, nsamp, GD):
                gb = list(range(g0, min(g0 + GD, nsamp)))
                st = {b: {} for b in gb}

                # -- column sums
                for b in gb:
                    B0, B1 = Bts[b]
                    srow_ps = pss.tile([1, 2 * C], f32, name=f"srow{b}",
                                       tag="sm1")
                    nc.tensor.matmul(srow_ps[:, 0:C], ones_t[0:128, :], B0[:],
                                     start=True, stop=False)
                    nc.tensor.matmul(srow_ps[:, 0:C], ones_t[0:M - 128, :],
                                     B1[:], start=False, stop=True)
                    st[b]["srow"] = srow_ps
                for b in gb:
                    s_sb = pool.tile([1, C], f32r, name=f"s_sb{b}", tag="s_sb",
                                     bufs=GD)
                    t_sb = pool.tile([1, C], f32r, name=f"t_sb{b}", tag="t_sb",
                                     bufs=GD)
                    nc.scalar.copy(s_sb[:], st[b]["srow"][:, 0:C])
                    nc.scalar.mul(t_sb[:], st[b]["srow"][:, 0:C], -1.0 / M)
                    st[b]["s"], st[b]["t"] = s_sb, t_sb
                    # squares for the trace (parallel to G):
                    # tr(G) = ||A||_F^2 - ||srow||^2 / M
                    sqB0 = pool.tile([128, C], f32r, name=f"sqB0_{b}",
                                     tag="sqB0", bufs=GD)
                    sqB1 = pool.tile([M - 128, C], f32r, name=f"sqB1_{b}",
                                     tag="sqB1", bufs=GD)
                    s2 = pool.tile([1, C], f32, name=f"s2_{b}", tag="s2",
                                   bufs=GD)
                    B0, B1 = Bts[b]
                    nc.scalar.activation(sqB0[:], B0[:],
                                         mybir.ActivationFunctionType.Square)
                    nc.scalar.activation(sqB1[:], B1[:],
                                         mybir.ActivationFunctionType.Square)
                    nc.scalar.activation(s2[:], st[b]["srow"][:, 0:C],
                                         mybir.ActivationFunctionType.Square)
                    st[b]["sqB0"], st[b]["sqB1"], st[b]["s2"] = sqB0, sqB1, s2
                for b in gb:
                    trrow_ps = pss.tile([1, 2 * C], f32, name=f"trrow{b}",
                                        tag="sm1")
                    nc.tensor.matmul(trrow_ps[:, 0:C], ones_t[0:128, :],
                                     st[b]["sqB0"][:], start=True, stop=False)
                    nc.tensor.matmul(trrow_ps[:, 0:C], ones_t[0:M - 128, :],
                                     st[b]["sqB1"][:], start=False, stop=True)
                    st[b]["trrow"] = trrow_ps
                for b in gb:
                    asum = pool.tile([1, 1], f32, name=f"asum{b}", tag="asum",
                                     bufs=GD)
                    nc.vector.tensor_reduce(out=asum[:],
                                            in_=st[b]["trrow"][:, 0:C],
                                            axis=mybir.AxisListType.X,
                                            op=AluOp.add)
                    s2sum = pool.tile([1, 1], f32, name=f"s2sum{b}", tag="s2s",
                                      bufs=GD)
                    nc.vector.tensor_reduce(out=s2sum[:], in_=st[b]["s2"][:],
                                            axis=mybir.AxisListType.X,
                                            op=AluOp.add)
                    tr_sb = pool.tile([1, 1], f32, name=f"tr_sb{b}",
                                      tag="tr_sb", bufs=GD)
                    nc.vector.scalar_tensor_tensor(
                        out=tr_sb[:], in0=s2sum[:], scalar=-1.0 / M,
                        in1=asum[:], op0=AluOp.mult, op1=AluOp.add)
                    inv_sb = pool.tile([1, 1], f32r, name=f"inv_sb{b}",
                                       tag="inv", bufs=GD)
                    nc.vector.reciprocal(inv_sb[:], tr_sb[:])
                    st[b]["invb"] = scalar_bcast(inv_sb, f"i{b}")

                # -- G = A^T A - M xbar xbar^T
                for b in gb:
                    B0, B1 = Bts[b]
                    GS = psg.tile([128, 2 * C], f32, name=f"GS{b}", tag="Yp")
                    for r in range(2):
                        nc.tensor.matmul(GS[:, C * r:C * (r + 1)],
                                         B0[:, 128 * r:128 * (r + 1)], B0[:],
                                         start=True, stop=False)
                        nc.tensor.matmul(GS[:, C * r:C * (r + 1)],
                                         B1[:, 128 * r:128 * (r + 1)], B1[:],
                                         start=False, stop=False)
                        nc.tensor.matmul(GS[:, C * r:C * (r + 1)],
                                         st[b]["t"][:, 128 * r:128 * (r + 1)],
                                         st[b]["s"][:], start=False, stop=True)
                    st[b]["G"] = GS

                # -- Y0 = G/trG ; T1 = 3I - Y0
                for b in gb:
                    Y0S = mats.tile([128, 2 * C], f32r, name=f"Y0S{b}",
                                    tag="Y0", bufs=GD)
                    nc.vector.tensor_scalar_mul(Y0S[:], st[b]["G"][:],
                                                st[b]["invb"][:])
                    st[b]["Y0"] = Y0S

                if stage == 1:
                    for r in range(2):
                        nc.sync.dma_start(
                            out[r, :, :],
                            st[gb[0]]["Y0"][:, C * r:C * (r + 1)].bitcast(f32))
                    break

                for b in gb:
                    T1S = mats.tile([128, 2 * C], f32r, name=f"T1S{b}",
                                    tag="T1", bufs=GD)
                    nc.vector.scalar_tensor_tensor(
                        out=T1S[:], in0=st[b]["Y0"][:], scalar=-1.0,
                        in1=threeIS_t[:], op0=AluOp.mult, op1=AluOp.add)
                    st[b]["T1"] = T1S

                # -- deferred-scale NS: materialize unscaled products and
                # fold the 0.5 factors into the 3I-minus-scaled-product ops.
                #   UY1 = Y0 T1            (Y1 = .5 UY1)
                #   T2  = 3I - .25 T1 UY1  (= 3I - Z1 Y1)
                #   UY2 = UY1 T2           (Y2 = .25 UY2)
                #   UZ2 = T2 T1            (Z2 = .25 UZ2)
                #   T3  = 3I - 1/16 UZ2 UY2
                #   Y3 ~ UY2 T3            (global scale irrelevant)
                for b in gb:
                    YpS = psg.tile([128, 2 * C], f32, name=f"YpS{b}", tag="Yp")
                    mmp(YpS, st[b]["Y0"], st[b]["T1"])
                    st[b]["Yp"] = YpS
                for b in gb:
                    Y1S = mats.tile([128, 2 * C], f32r, name=f"Y1S{b}",
                                    tag="Y1", bufs=GD)
                    nc.scalar.copy(Y1S[:], st[b]["Yp"][:])
                    st[b]["Y1"] = Y1S

                # -- iter2
                for b in gb:
                    PpS = psg.tile([128, 2 * C], f32, name=f"PpS{b}", tag="Yp")
                    mmp(PpS, st[b]["T1"], st[b]["Y1"])
                    st[b]["Pp"] = PpS
                for b in gb:
                    T2S = mats.tile([128, 2 * C], f32r, name=f"T2S{b}",
                                    tag="T", bufs=GD)
                    nc.vector.scalar_tensor_tensor(
                        out=T2S[:], in0=st[b]["Pp"][:], scalar=-0.25,
                        in1=threeIS_t[:], op0=AluOp.mult, op1=AluOp.add)
                    st[b]["T2"] = T2S
                for b in gb:
                    Yp2S = psg.tile([128, 2 * C], f32, name=f"Yp2S{b}",
                                    tag="Yp")
                    mmp(Yp2S, st[b]["Y1"], st[b]["T2"])
                    st[b]["Yp2"] = Yp2S
                for b in gb:
                    Y2S = mats.tile([128, 2 * C], f32r, name=f"Y2S{b}",
                                    tag="Y2", bufs=GD)
                    nc.scalar.copy(Y2S[:], st[b]["Yp2"][:])
                    st[b]["Y2"] = Y2S
                for b in gb:
                    ZpS = psg.tile([128, 2 * C], f32, name=f"ZpS{b}", tag="Yp")
                    mmp(ZpS, st[b]["T2"], st[b]["T1"])
                    st[b]["Zp"] = ZpS
                for b in gb:
                    Z2S = mats.tile([128, 2 * C], f32r, name=f"Z2S{b}",
                                    tag="Z", bufs=GD)
                    nc.scalar.copy(Z2S[:], st[b]["Zp"][:])
                    st[b]["Z2"] = Z2S

                # -- iter3 (Z dead)
                for b in gb:
                    Pp3S = psg.tile([128, 2 * C], f32, name=f"Pp3S{b}",
                                    tag="Yp")
                    mmp(Pp3S, st[b]["Z2"], st[b]["Y2"])
                    st[b]["Pp3"] = Pp3S
                for b in gb:
                    T3S = mats.tile([128, 2 * C], f32r, name=f"T3S{b}",
                                    tag="T", bufs=GD)
                    nc.vector.scalar_tensor_tensor(
                        out=T3S[:], in0=st[b]["Pp3"][:], scalar=-1.0 / 16.0,
                        in1=threeIS_t[:], op0=AluOp.mult, op1=AluOp.add)
                    st[b]["T3"] = T3S
                for b in gb:
                    Y3pS = psg.tile([128, 2 * C], f32, name=f"Y3pS{b}",
                                    tag="Yp")
                    mmp(Y3pS, st[b]["Y2"], st[b]["T3"])
                    st[b]["Y3p"] = Y3pS

                # -- flat-normalize + staging
                for b in gb:
                    sqS = pool.tile([128, 2 * C], f32r, name=f"sqS{b}",
                                    tag="scr", bufs=GD)
                    nc.scalar.activation(sqS[:], st[b]["Y3p"][:],
                                         mybir.ActivationFunctionType.Square)
                    st[b]["sq"] = sqS
                for b in gb:
                    ssqrow_ps = pss.tile([1, 2 * C], f32, name=f"ssqrow{b}",
                                         tag="sm1")
                    nc.tensor.matmul(ssqrow_ps[:], ones_t[0:128, :],
                                     st[b]["sq"][:], start=True, stop=True)
                    st[b]["ssqrow"] = ssqrow_ps
                for b in gb:
                    ssq_sb = pool.tile([1, 1], f32, name=f"ssq_sb{b}",
                                       tag="tr_sb", bufs=GD)
                    nc.vector.tensor_reduce(out=ssq_sb[:],
                                            in_=st[b]["ssqrow"][:],
                                            axis=mybir.AxisListType.X,
                                            op=AluOp.add)
                    sqr_sb = pool.tile([1, 1], f32, name=f"sqr_sb{b}",
                                       tag="sqr", bufs=GD)
                    nc.scalar.sqrt(sqr_sb[:], ssq_sb[:])
                    rsq_sb = pool.tile([1, 1], f32r, name=f"rsq_sb{b}",
                                       tag="inv", bufs=GD)
                    nc.vector.reciprocal(rsq_sb[:], sqr_sb[:])
                    st[b]["rsqb"] = scalar_bcast(rsq_sb, f"r{b}")
                for b in gb:
                    FS = mats.tile([128, 2 * C], f32r, name=f"FS{b}", tag="F",
                                   bufs=GD)
                    nc.vector.tensor_scalar_mul(FS[:], st[b]["Y3p"][:],
                                                st[b]["rsqb"][:])
                    st[b]["F"] = FS

                if stage == 2:
                    for r in range(2):
                        nc.sync.dma_start(
                            out[r, :, :],
                            st[gb[0]]["F"][:, C * r:C * (r + 1)].bitcast(f32))
                    break

                for b in gb:
                    for hh in range(2):
                        nc.sync.dma_start(
                            a2a_in_v[b // (BL // 2)][hh, b % (BL // 2)],
                            st[b]["F"][:, C * hh:C * (hh + 1)]
                                .rearrange("p (j i) -> p j i", j=8, i=32))

            # ---------- AllToAll (two halves; first overlaps pooling) ----
            if stage >= 3:
                for q in range(2):
                    nc.gpsimd.collective_compute(
                        "AllToAll", AluOp.bypass, replica_groups=rg,
                        ins=[a2a_in[q].opt()], outs=[a2a_out[q].opt()])

            if stage == 3:
                tmp = bigpool.tile([128, 4096], f32, name="tmp")
                for q in range(2):
                    nc.sync.dma_start(tmp[:, 2048 * q:2048 * (q + 1)],
                                      a2a_out[q][:].bitcast(f32))
                nc.sync.dma_start(out[:], tmp[:])

            if stage >= 4:
                # ------- consumer: BIG [128, 4096], free = [h, s, b, i] -------
                BIG = bigpool.tile([128, 2 * 8 * BL * 32], f32r, name="BIG")
                a2a_out_v = [a2a_out[q].flatten().rearrange(
                    "(s h p b i) -> h s p b i", s=8, h=2, p=128, b=BL // 2,
                    i=32) for q in range(2)]
                BIG_v = BIG[:].rearrange("p (h s q b i) -> q h s p b i",
                                         h=2, s=8, q=2, b=BL // 2, i=32)
                for q in range(2):
                    for hh in range(2):
                        for s in range(8):
                            nc.sync.dma_start(BIG_v[q, hh, s],
                                              a2a_out_v[q][hh, s])

                # ------- projection: EMB[64, 512], W streamed -------
                EMB = pss.tile([64, E], f32, name="EMB", tag="sm1")
                BIG_k = BIG[:].rearrange("p (h sb i) -> h i p sb",
                                         h=2, sb=64, i=32)
                wT_v = wT.rearrange("(c p) e -> c p e", p=128)  # [64,128,512]
                for c in range(NCH):
                    i_local, hh = c // 2, c % 2
                    wq = wpool.tile([128, E], f32r, name=f"wq{c}", tag="wq",
                                    bufs=36)
                    nc.sync.dma_start(wq[:], wT_v[c])
                    nc.tensor.matmul(
                        EMB[:], BIG_k[hh, i_local], wq[:],
                        start=(c == 0), stop=(c == NCH - 1))

                emb_sb = pool.tile([64, E], f32, name="emb_sb", tag="emb", bufs=1)
                nc.vector.tensor_copy(emb_sb[:], EMB[:])
                if stage == 4:
                    nc.sync.dma_start(out[:], emb_sb[:])

            if stage >= 5:
                nc.sync.dma_start(rs_in[:], emb_sb[:])

                # ------- ReduceScatter: [64, E] -> [8, E] -------
                nc.gpsimd.collective_compute(
                    "ReduceScatter", AluOp.add, replica_groups=rg,
                    ins=[rs_in.opt()], outs=[rs_out.opt()])

                # ------- BN fold + final L2 normalize -------
                e_sb = pool.tile([BL, E], f32, name="e_sb", tag="fin", bufs=1)
                nc.sync.dma_start(e_sb[:], rs_out[:])
                e_bn = pool.tile([BL, E], f32, name="e_bn", tag="fin2", bufs=1)
                nc.vector.tensor_tensor(e_bn[:], e_sb[:], bnsc_t[:], AluOp.mult)
                nc.vector.tensor_tensor(e_bn[:], e_bn[:], bnsh_t[:], AluOp.add)
                scr3 = pool.tile([BL, E], f32, name="scr3", tag="fin", bufs=1)
                nrm_sb = pool.tile([BL, 1], f32, name="nrm_sb", tag="nrm")
                nc.scalar.activation(
                    scr3[:], e_bn[:], mybir.ActivationFunctionType.Square,
                    accum_out=nrm_sb[:])
                nrms_sb = pool.tile([BL, 1], f32, name="nrms_sb", tag="nrms")
                nc.scalar.sqrt(nrms_sb[:], nrm_sb[:])
                rs_sb = pool.tile([BL, 1], f32, name="rs_sb", tag="nrmr")
                nc.vector.reciprocal(rs_sb[:], nrms_sb[:])
                e_fin = pool.tile([BL, E], f32, name="e_fin", tag="fin3", bufs=1)
                nc.vector.tensor_scalar_mul(e_fin[:], e_bn[:], rs_sb[:])
                nc.sync.dma_start(out[:], e_fin[:])

    _split_excess_waits(nc)
    return nc


def host_inputs(feat, W_proj, b_proj, bn_gamma, bn_beta, bn_mean, bn_var):
    """Build the 8 per-core input maps."""
    feat = np.ascontiguousarray(np.asarray(feat, dtype=np.float32))
    W_proj = np.asarray(W_proj, dtype=np.float32)
    featT = feat.reshape(B, C, M).transpose(0, 2, 1)          # [64, 196, 256]
    bnscale = (np.asarray(bn_gamma) /
               np.sqrt(np.asarray(bn_var) + BN_EPS)).astype(np.float32)
    bnshift = ((np.asarray(b_proj) - np.asarray(bn_mean)) * bnscale
               + np.asarray(bn_beta)).astype(np.float32)
    bnsc_rep = np.ascontiguousarray(np.broadcast_to(bnscale, (BL, E)))
    bnsh_rep = np.ascontiguousarray(np.broadcast_to(bnshift, (BL, E)))

    onesc = np.ones((128, 1), np.float32)
    onesr = np.ones((1, 128), np.float32)
    threeIS = np.zeros((128, 2 * C), np.float32)
    threeIS[:, 0:128] = 3.0 * np.eye(128, dtype=np.float32)
    threeIS[:, C + 128:C + 256] = 3.0 * np.eye(128, dtype=np.float32)

    in_maps = []
    for i in range(N_CORES):
        in_maps.append({
            "featT": np.ascontiguousarray(featT[i * BL:(i + 1) * BL]),
            "wT": np.ascontiguousarray(W_proj[:, KL * i:KL * (i + 1)].T),
            "onesc": onesc, "onesr": onesr, "threeIS": threeIS,
            "bnsc": bnsc_rep, "bnsh": bnsh_rep,
        })
    return in_maps


def kernel(feat, W_proj, b_proj, bn_gamma, bn_beta, bn_mean, bn_var):
    if "nc" not in _cache:
        _cache["nc"] = _build()
    nc = _cache["nc"]
    in_maps = host_inputs(feat, W_proj, b_proj, bn_gamma, bn_beta,
                          bn_mean, bn_var)
    last_err = None
    for _attempt in range(4):
        try:
            res = run_bass_kernel_spmd(nc, in_maps,
                                       core_ids=list(range(N_CORES)))
            break
        except Exception as e:  # transient NRT_EXEC_UNIT_UNRECOVERABLE flakes
            last_err = e
            import time as _time
            _time.sleep(2.0)
    else:
        raise last_err
    return np.concatenate([res.results[i]["out"] for i in range(N_CORES)],
                          axis=0)
